# revision 4
# baseline (speedup 1.0000x reference)
"""Trainium2 Bass kernel for the multi-level hash-grid context layer.

Algorithm (corner-stream): for the instant-NGP neighbor structure,
neighbor k of entry i equals the corner hash of entry i+s_k for 8 fixed
shifts s_k = {0,1,R,R+1,R^2,R^2+1,R^2+R,R^2+R+1}. So instead of 8 random
gathers per entry we gather ONE corner stream g[j] = x[nbr0[j]] (+ a small
structure-derived extension) and compute the 8-neighbor sum with shifted
streaming adds. Host-side verification compares every entry's actual
neighbor indices against the streamed relation; mismatches (level
boundaries / clamps / arbitrary inputs) are fixed up with direct gathers.

Gather engine: InstDMAGatherAnt (bulk 256B-block gather, int16 indices)
from a bf16 copy of the table; sub-row (16 bf16 rows per 256B block)
extracted on-chip with one-hot masks + reduce. Dense levels (R^3 <= 2^19)
skip the gather entirely (corner stream is the identity there).

All 8 cores run one uniform program (SPMD); each core owns 1/8 of every
level. Work is data-parameterized per core.
"""
import numpy as np
import ml_dtypes

import concourse.bass as bass
import concourse.bacc as bacc
import concourse.mybir as mybir
from concourse.tile import TileContext, add_dep_helper
from concourse.bass_utils import run_bass_kernel_spmd

RES = [16, 20, 25, 32, 40, 51, 64, 81, 102, 128, 161, 203, 256, 323, 406, 512]
CAP = 1 << 19
PRIMES = np.array([1, 2654435761, 805459861], dtype=np.uint32)
NC = 8           # cores
P = 128          # partitions
CT = 64          # gather positions per partition per tile
CB = 192         # phase-B chunk columns (entries per partition per chunk)
BPB = 16         # bf16 rows per 256B gather block
TWO_STREAM_MIN_R = 300

_bf16 = ml_dtypes.bfloat16


def _levels():
    sizes = [min(r ** 3, CAP) for r in RES]
    offs = np.concatenate([[0], np.cumsum(sizes)]).astype(np.int64)
    out = []
    for i, r in enumerate(RES):
        out.append(dict(R=r, T=sizes[i], off=int(offs[i]), dense=r ** 3 <= CAP,
                        chunk=-(-sizes[i] // NC)))
    return out, int(offs[-1])


def _ext_idx(lv, count):
    R = lv["R"]
    j = np.arange(lv["T"], lv["T"] + count, dtype=np.int64)
    cx, cy, cz = (j // (R * R)) % R, (j // R) % R, j % R
    h = (cx.astype(np.uint32) * PRIMES[0]) ^ (cy.astype(np.uint32) * PRIMES[1]) ^ \
        (cz.astype(np.uint32) * PRIMES[2])
    return (lv["off"] + (h % np.uint32(CAP)).astype(np.int64)).astype(np.int64)


def _plan(neighbor_idx):
    levels, N = _levels()
    for lv in levels:
        off, T, R = lv["off"], lv["T"], lv["R"]
        nbr = neighbor_idx[off:off + T]
        E = R * R + R + 2
        g = np.empty(T + E, dtype=np.int64)
        if lv["dense"]:
            g[:] = off + np.arange(T + E, dtype=np.int64)
        else:
            g[:T] = nbr[:, 0]
            g[T:] = _ext_idx(lv, E)
        lv["g_idx"] = g
        ok = np.ones(T, dtype=bool)
        for k, s in enumerate([0, 1, R, R + 1, R * R, R * R + 1, R * R + R, R * R + R + 1]):
            ok &= nbr[:, k] == g[s:s + T]
        # for non-dense levels, the streamed g values must also lie within the
        # level (the bf16 gather window is this level only)
        if not lv["dense"]:
            inlvl = (g >= off) & (g < off + T)
            for s in [0, 1, R, R + 1, R * R, R * R + 1, R * R + R, R * R + R + 1]:
                ok &= inlvl[s:s + T]
        lv["ok"] = ok
        lv["E"] = E

    # per-level segment meta (uniform across cores)
    segs = []
    for li, lv in enumerate(levels):
        PL = -(-lv["chunk"] // P)
        mode = "dense" if lv["dense"] else ("two" if lv["R"] >= TWO_STREAM_MIN_R else "one")
        segs.append(dict(li=li, R=lv["R"], PL=PL, mode=mode,
                         off=lv["off"], T=lv["T"], chunk=lv["chunk"]))

    # fixups per (core, level) and hard fixups (neighbors outside own level)
    fix = [[[] for _ in levels] for _ in range(NC)]
    hard = [[] for _ in range(NC)]
    for li, lv in enumerate(levels):
        off, T = lv["off"], lv["T"]
        bad = np.nonzero(~lv["ok"])[0]
        if len(bad) == 0:
            continue
        nb = neighbor_idx[off + bad]  # [nbad, 8]
        # fixup rows must come from this level's bf16 window (dense: own level
        # f32 path also gathers from the level's bf16 window)
        ok_here = ((nb >= off) & (nb < off + T)).all(axis=1)
        for j, oh in zip(bad, ok_here):
            c = min(int(j // lv["chunk"]), NC - 1)
            if oh:
                fix[c][li].append(int(j))
            else:
                hard[c].append(off + int(j))

    # uniform fixup quotas per level
    fq = []
    for li in range(len(levels)):
        mx = max(len(fix[c][li]) for c in range(NC))
        fq.append(-(-max(mx, 1) // P) * P)
    nhard_max = max(len(h) for h in hard)
    HQ = -(-max(nhard_max, 0) // P) * P  # hard quota (0 if none)

    # gather stream layout per level (positions in "w" space), uniform:
    #   one:   [0, 128*PL + E)            main window
    #   two:   [0, L0) main, [L0, L0+L1) +R^2 window
    #   dense: no main window
    # then fixup region: 8 * fq[li] positions, laid out per-partition:
    #   partition p, slot j, neighbor k -> fixoff + p*(q*8) + j*8 + k
    goff = 0
    for sm in segs:
        R, PL, li = sm["R"], sm["PL"], sm["li"]
        sm["g0"] = goff
        if sm["mode"] == "one":
            sm["len0"] = P * PL + R * R + R + 2
            sm["len1"] = 0
        elif sm["mode"] == "two":
            sm["len0"] = P * PL + R + 2
            sm["len1"] = P * PL + R + 2
        else:
            sm["len0"] = 0
            sm["len1"] = 0
        sm["fixoff"] = sm["len0"] + sm["len1"]
        sm["q"] = fq[li] // P
        slen = sm["fixoff"] + 8 * fq[li]
        slen = -(-slen // (P * CT)) * (P * CT)  # pad to gather-tile multiple
        sm["slen"] = slen
        sm["ntiles"] = slen // (P * CT)
        goff += slen
    GTOT = goff  # gbuf rows per core

    # out layout
    ooff = 0
    for sm in segs:
        sm["o0"] = ooff
        ooff += P * sm["PL"]
    OUT_ROWS = ooff + P  # + dummy tail for fixup padding + hard pads

    return dict(levels=levels, segs=segs, fix=fix, hard=hard, fq=fq, HQ=HQ,
                GTOT=GTOT, OUT_ROWS=OUT_ROWS, N=N)


def _core_arrays(plan, neighbor_idx, c):
    """Build per-core gather idx (int16 blocks), masks (bf16), scatter idx."""
    segs, levels = plan["segs"], plan["levels"]
    rows = np.zeros(plan["GTOT"], dtype=np.int64)   # global row per position
    valid = np.zeros(plan["GTOT"], dtype=bool)
    scat = []
    for sm in segs:
        lv = levels[sm["li"]]
        off, T, R = lv["off"], lv["T"], sm["R"]
        es = c * sm["chunk"]
        g = lv["g_idx"]
        base = sm["g0"]

        def put(dst, start, length):
            s = max(0, min(start, len(g)))
            e = max(0, min(start + length, len(g)))
            if e > s:
                rows[dst + (s - start): dst + (e - start)] = g[s:e]
                valid[dst + (s - start): dst + (e - start)] = True

        if sm["mode"] == "one":
            put(base, es, sm["len0"])
        elif sm["mode"] == "two":
            put(base, es, sm["len0"])
            put(base + sm["len0"], es + R * R, sm["len1"])
        # fixups
        fxs = plan["fix"][c][sm["li"]]
        q = sm["q"]
        soff = np.full((P, q), plan["OUT_ROWS"] - 1, dtype=np.int32)
        for f, j in enumerate(fxs):
            p, jj = f // q, f % q
            w = base + sm["fixoff"] + p * (q * 8) + jj * 8
            rows[w:w + 8] = neighbor_idx[off + j]
            valid[w:w + 8] = True
            soff[p, jj] = sm["o0"] + j
        scat.append(soff)
        # positions of this level must gather within [off, off+T): map invalid
        # to off (block 0)
        lo, hi = base, base + sm["slen"]
        r = rows[lo:hi]
        v = valid[lo:hi]
        r[~v] = off
        np.clip(r, off, off + T - 1, out=r)
        rows[lo:hi] = r

    # per-level: block idx (within level) + sub code, then interleave-feed order
    gidx = np.zeros((plan["GTOT"] // (P * CT), P, CT * 8), dtype=np.int16)
    msk = np.zeros((plan["GTOT"] // (P * CT), P, CT * 16), dtype=_bf16)
    tglob = 0
    for sm in segs:
        lv = levels[sm["li"]]
        lo = sm["g0"]
        GL = sm["slen"] // P
        r = rows[lo:lo + sm["slen"]] - lv["off"]
        blk = (r // BPB).astype(np.int16)
        sub = (r % BPB).astype(np.int16)
        blk_m = blk.reshape(P, GL)
        sub_m = sub.reshape(P, GL)
        for t in range(sm["ntiles"]):
            bt = blk_m[:, t * CT:(t + 1) * CT]          # [P, CT] output-layout
            feed = bt.T.reshape(-1)                     # feed order: i -> (p=i%128, c=i//128)
            w = feed.reshape(CT * 8, 16).T              # wrapped [16, n/16]
            gidx[tglob, :, :] = np.tile(w, (8, 1))
            st = sub_m[:, t * CT:(t + 1) * CT]          # [P, CT]
            m = np.zeros((P, CT, 16), dtype=_bf16)
            np.put_along_axis(m, st[:, :, None].astype(np.int64), _bf16(1.0), axis=2)
            msk[tglob] = m.reshape(P, CT * 16)
            tglob += 1

    scat_all = np.concatenate([s.reshape(P, -1) for s in scat], axis=1)  # [P, sum q]
    # hard fixups
    hq = plan["HQ"]
    hrows = np.zeros((max(hq, 1), 8), dtype=np.int32)
    hout = np.full(max(hq, 1), plan["OUT_ROWS"] - 1, dtype=np.int32)
    segs_by_entry = plan["hard"][c]
    for f, ge in enumerate(segs_by_entry):
        hrows[f] = neighbor_idx[ge]
        # locate out row: level + local j
        for sm in segs:
            lv = levels[sm["li"]]
            if lv["off"] <= ge < lv["off"] + lv["T"]:
                j = ge - lv["off"] - c * sm["chunk"]
                hout[f] = sm["o0"] + j
                break
    return gidx, msk, scat_all.astype(np.int32), hrows, hout


def _build_nc(plan, NT, NQS, dense_rows):
    """Build the uniform Bass program."""
    segs = plan["segs"]
    nc = bacc.Bacc("TRN2", target_bir_lowering=False, debug=False, num_devices=NC)
    f32, bf16, i16, i32 = (mybir.dt.float32, mybir.dt.bfloat16,
                           mybir.dt.int16, mybir.dt.int32)
    N = plan["N"]
    xb = nc.dram_tensor("xb", [N, 8], bf16, kind="ExternalInput")
    xd = nc.dram_tensor("xd", [dense_rows, 8], f32, kind="ExternalInput")
    gidx = nc.dram_tensor("gidx", [NT, P, CT * 8], i16, kind="ExternalInput")
    mskd = nc.dram_tensor("mskd", [NT, P, CT * 16], bf16, kind="ExternalInput")
    wti = nc.dram_tensor("wt", [P, 64], f32, kind="ExternalInput")
    bti = nc.dram_tensor("bt", [P, 8], f32, kind="ExternalInput")
    scat = nc.dram_tensor("scat", [P, NQS], i32, kind="ExternalInput")
    HQ = plan["HQ"]
    if HQ:
        hrowst = nc.dram_tensor("hrows", [HQ, 8], i32, kind="ExternalInput")
        houtt = nc.dram_tensor("hout", [HQ], i32, kind="ExternalInput")
    out = nc.dram_tensor("out", [plan["OUT_ROWS"], 8], f32, kind="ExternalOutput")
    nfx_cols = NQS * 8 + (plan["HQ"] // P) * 8
    fxo = nc.dram_tensor("fxo", [P, max(nfx_cols, 8)], f32, kind="ExternalOutput")
    gbuf = nc.dram_tensor("gbuf", [plan["GTOT"] * 8], f32)

    xbf = xb.ap().rearrange("a b -> (a b)")
    gb = gbuf.ap()
    xdf = xd.ap().rearrange("a b -> (a b)")
    outf = out.ap().rearrange("a b -> (a b)")

    with TileContext(nc) as tc:
        with (
            tc.tile_pool(name="const", bufs=1) as constp,
            tc.tile_pool(name="pa", bufs=2) as pa,
            tc.tile_pool(name="pb", bufs=2) as pb,
            tc.tile_pool(name="pbt", bufs=2) as pbt,
        ):
            wt = constp.tile([P, 64], f32)
            bt = constp.tile([P, 8], f32)
            nc.sync.dma_start(out=wt[:], in_=wti[:])
            nc.sync.dma_start(out=bt[:], in_=bti[:])

            # ---- Phase A: gather + extract per level ----
            for sm in segs:
                if sm["ntiles"] == 0:
                    continue
                lv = plan["levels"][sm["li"]]
                nblk = -(-lv["T"] // BPB)
                win = bass.AP(xb, lv["off"] * 8, [[128, nblk], [1, 128]])
                GL = sm["slen"] // P
                for t in range(sm["ntiles"]):
                    tg = sm["tile_base"] + t
                    idx_sb = pa.tile([P, CT * 8], i16, tag="idx")
                    nc.sync.dma_start(out=idx_sb[:], in_=gidx[tg])
                    mk = pa.tile([P, CT * 16], bf16, tag="msk")
                    nc.sync.dma_start(out=mk[:], in_=mskd[tg])
                    gat = pa.tile([P, CT * 128], bf16, tag="gat")
                    nc.gpsimd.dma_gather(
                        out_ap=gat[:].rearrange("p (c e) -> p c e", e=128),
                        in_ap=win,
                        idxs_ap=idx_sb[:],
                        num_idxs=P * CT,
                        num_idxs_reg=P * CT,
                        elem_size=128,
                        single_packet=False,
                    )
                    tmp = pa.tile([P, CT * 128], bf16, tag="tmp")
                    in0 = gat[:].rearrange("p (c s e) -> p c s e", s=16, e=8)
                    in1 = mk[:].rearrange("p (c s) -> p c s", s=16)
                    in1 = bass.AP(in1.tensor, in1.offset, in1.ap + [[0, 8]])
                    outv = bass.AP(tmp[:].tensor, tmp[:].offset,
                                   [tmp[:].ap[0], [128, CT], [1, 16], [16, 8]])
                    nc.vector.tensor_tensor(out=outv, in0=in0, in1=in1,
                                            op=mybir.AluOpType.mult)
                    rows_t = pa.tile([P, CT * 8], f32, tag="rows")
                    nc.vector.reduce_sum(
                        out=rows_t[:],
                        in_=tmp[:].rearrange("p (ce s) -> p ce s", s=16),
                        axis=mybir.AxisListType.X)
                    dst = bass.AP(gbuf, (sm["g0"] + t * CT) * 8,
                                  [[GL * 8, P], [1, CT * 8]])
                    nc.sync.dma_start(out=dst, in_=rows_t[:])

            # ---- Phase B: streaming shifted sums + affine ----
            out_writes = {}
            for sm in segs:
                R, PL = sm["R"], sm["PL"]
                lv = plan["levels"][sm["li"]]
                if sm["mode"] == "dense":
                    src, sbase = xd, (lv["off"] + 0) * 8
                    # per-core entry start offset added via es
                else:
                    src, sbase = gbuf, sm["g0"] * 8
                nchunk = -(-PL // CB)
                for k in range(nchunk):
                    w = min(CB, PL - k * CB)
                    WN = w + R + 2
                    t0 = pbt.tile([P, WN * 8], f32, tag="t")
                    t1 = pbt.tile([P, WN * 8], f32, tag="t")
                    if sm["mode"] == "two":
                        a0 = bass.AP(src, sbase + k * CB * 8,
                                     [[PL * 8, P], [1, WN * 8]])
                        a1 = bass.AP(src, (sm["g0"] + sm["len0"] + k * CB) * 8,
                                     [[PL * 8, P], [1, WN * 8]])
                    else:
                        a0 = bass.AP(src, sbase + k * CB * 8,
                                     [[PL * 8, P], [1, WN * 8]])
                        a1 = bass.AP(src, sbase + (k * CB + R * R) * 8,
                                     [[PL * 8, P], [1, WN * 8]])
                    nc.sync.dma_start(out=t0[:], in_=a0)
                    nc.sync.dma_start(out=t1[:], in_=a1)
                    # P = t0 + t1 (in place), u = P[j]+P[j+1] (into t1), y = u[j]+u[j+R]
                    nc.vector.tensor_tensor(out=t0[:], in0=t0[:], in1=t1[:],
                                            op=mybir.AluOpType.add)
                    UN = w + R
                    nc.vector.tensor_tensor(out=t1[:, :UN * 8],
                                            in0=t0[:, :UN * 8], in1=t0[:, 8:(UN + 1) * 8],
                                            op=mybir.AluOpType.add)
                    y = pb.tile([P, CB * 8], f32, tag="y")
                    nc.vector.tensor_tensor(out=y[:, :w * 8], in0=t1[:, :w * 8],
                                            in1=t1[:, R * 8:(w + R) * 8],
                                            op=mybir.AluOpType.add)
                    # affine: out = (y/8) @ W'^T + b, W' pre-scaled host-side
                    ybf = pb.tile([P, CB * 8], bf16, tag="ybf")
                    nc.vector.tensor_copy(out=ybf[:, :w * 8], in_=y[:, :w * 8])
                    ot = pb.tile([P, CB * 8], f32, tag="ot")
                    for h in range(2):
                        prod = pb.tile([P, CB * 32], bf16, tag="prod")
                        i0 = ybf[:, :w * 8].rearrange("p (c e) -> p c e", e=8)
                        i0 = bass.AP(i0.tensor, i0.offset,
                                     [i0.ap[0], i0.ap[1], [0, 4], i0.ap[2]])
                        wv = wt[:, h * 32:(h + 1) * 32].rearrange("p (o e) -> p o e", e=8)
                        i1 = bass.AP(wv.tensor, wv.offset,
                                     [wv.ap[0], [0, w], wv.ap[1], wv.ap[2]])
                        pv = prod[:, :w * 32].rearrange("p (c o e) -> p c o e", o=4, e=8)
                        nc.vector.tensor_tensor(out=pv, in0=i0, in1=i1,
                                                op=mybir.AluOpType.mult)
                        nc.vector.reduce_sum(
                            out=bass.AP(ot[:].tensor, ot[:].offset + h * 4,
                                    [ot[:].ap[0], [8, w], [1, 4]]),
                            in_=prod[:, :w * 32].rearrange("p (co e) -> p co e", e=8),
                            axis=mybir.AxisListType.X)
                    bv = bass.AP(bt[:].tensor, bt[:].offset,
                                 [bt[:].ap[0], [0, w], [1, 8]])
                    ov = ot[:, :w * 8].rearrange("p (c e) -> p c e", e=8)
                    nc.vector.tensor_tensor(out=ov, in0=ov, in1=bv,
                                            op=mybir.AluOpType.add)
                    od = bass.AP(out, (sm["o0"] + k * CB) * 8,
                                 [[PL * 8, P], [1, w * 8]])
                    wi = nc.sync.dma_start(out=od, in_=ot[:, :w * 8])
                    out_writes.setdefault(sm["li"], []).append(wi)

            # ---- Phase C: fixups (scatter corrected rows) ----
            qpos = 0
            for sm in segs:
                q = sm["q"]
                if q == 0:
                    continue
                GLb = sm["slen"] // P
                fr = pb.tile([P, q * 64], f32, tag="fr")
                a = bass.AP(gbuf, (sm["g0"] + sm["fixoff"]) * 8,
                            [[q * 64, P], [1, q * 64]])
                nc.sync.dma_start(out=fr[:], in_=a)
                acc = pb.tile([P, q * 8], f32, tag="facc")
                v = fr[:].rearrange("p (j k e) -> p j k e", k=8, e=8)
                nc.vector.tensor_copy(out=acc[:].rearrange("p (j e) -> p j e", e=8),
                                      in_=v[:, :, 0, :])
                for kk in range(1, 8):
                    nc.vector.tensor_tensor(
                        out=acc[:].rearrange("p (j e) -> p j e", e=8),
                        in0=acc[:].rearrange("p (j e) -> p j e", e=8),
                        in1=v[:, :, kk, :], op=mybir.AluOpType.add)
                # affine on acc
                accb = pb.tile([P, q * 8], bf16, tag="faccb")
                nc.vector.tensor_copy(out=accb[:], in_=acc[:])
                facc = pb.tile([P, q * 8], f32, tag="fout")
                for h in range(2):
                    prod = pb.tile([P, q * 32], bf16, tag="fprod")
                    i0 = accb[:].rearrange("p (c e) -> p c e", e=8)
                    i0 = bass.AP(i0.tensor, i0.offset, [i0.ap[0], i0.ap[1], [0, 4], i0.ap[2]])
                    wv = wt[:, h * 32:(h + 1) * 32].rearrange("p (o e) -> p o e", e=8)
                    i1 = bass.AP(wv.tensor, wv.offset, [wv.ap[0], [0, q], wv.ap[1], wv.ap[2]])
                    pv = prod[:].rearrange("p (c o e) -> p c o e", o=4, e=8)
                    nc.vector.tensor_tensor(out=pv, in0=i0, in1=i1, op=mybir.AluOpType.mult)
                    nc.vector.reduce_sum(
                        out=bass.AP(facc[:].tensor, facc[:].offset + h * 4,
                                    [facc[:].ap[0], [8, q], [1, 4]]),
                        in_=prod[:].rearrange("p (co e) -> p co e", e=8),
                        axis=mybir.AxisListType.X)
                bv = bass.AP(bt[:].tensor, bt[:].offset, [bt[:].ap[0], [0, q], [1, 8]])
                fv = facc[:].rearrange("p (c e) -> p c e", e=8)
                nc.vector.tensor_tensor(out=fv, in0=fv, in1=bv, op=mybir.AluOpType.add)
                nc.sync.dma_start(out=fxo[:, qpos * 8:(qpos + q) * 8], in_=facc[:])
                qpos += q

            # ---- hard fixups (rare; neighbors cross levels) ----
            if HQ:
                nh = HQ // P
                hi = pb.tile([P, nh * 8], i32, tag="hi")
                nc.sync.dma_start(out=hi[:], in_=hrowst.ap().rearrange(
                    "(a p) b -> p (a b)", p=P))
                hacc = pb.tile([P, nh * 8], f32, tag="hacc")
                hrow = pb.tile([P, 8], bf16, tag="hrow")
                for j in range(nh):
                    for kk in range(8):
                        nc.gpsimd.indirect_dma_start(
                            out=hrow[:], out_offset=None, in_=xb.ap(),
                            in_offset=bass.IndirectOffsetOnAxis(
                                ap=hi[:, j * 8 + kk:j * 8 + kk + 1], axis=0))
                        if kk == 0:
                            nc.vector.tensor_copy(out=hacc[:, j * 8:(j + 1) * 8], in_=hrow[:])
                        else:
                            nc.vector.tensor_tensor(
                                out=hacc[:, j * 8:(j + 1) * 8],
                                in0=hacc[:, j * 8:(j + 1) * 8], in1=hrow[:],
                                op=mybir.AluOpType.add)
                # affine via 64 MACs would bloat; reuse broadcast trick
                haccb = pb.tile([P, nh * 8], bf16, tag="haccb")
                nc.vector.tensor_copy(out=haccb[:], in_=hacc[:])
                hfin = pb.tile([P, nh * 8], f32, tag="hfin")
                for h in range(2):
                    prod = pb.tile([P, nh * 32], bf16, tag="hprod")
                    i0 = haccb[:].rearrange("p (c e) -> p c e", e=8)
                    i0 = bass.AP(i0.tensor, i0.offset, [i0.ap[0], i0.ap[1], [0, 4], i0.ap[2]])
                    wv = wt[:, h * 32:(h + 1) * 32].rearrange("p (o e) -> p o e", e=8)
                    i1 = bass.AP(wv.tensor, wv.offset, [wv.ap[0], [0, nh], wv.ap[1], wv.ap[2]])
                    pv = prod[:].rearrange("p (c o e) -> p c o e", o=4, e=8)
                    nc.vector.tensor_tensor(out=pv, in0=i0, in1=i1, op=mybir.AluOpType.mult)
                    nc.vector.reduce_sum(
                        out=bass.AP(hfin[:].tensor, hfin[:].offset + h * 4,
                                    [hfin[:].ap[0], [8, nh], [1, 4]]),
                        in_=prod[:].rearrange("p (co e) -> p co e", e=8),
                        axis=mybir.AxisListType.X)
                bv = bass.AP(bt[:].tensor, bt[:].offset, [bt[:].ap[0], [0, nh], [1, 8]])
                hv = hfin[:].rearrange("p (c e) -> p c e", e=8)
                nc.vector.tensor_tensor(out=hv, in0=hv, in1=bv, op=mybir.AluOpType.add)
                nc.sync.dma_start(out=fxo[:, NQS * 8:(NQS + nh) * 8], in_=hfin[:])
    nc.compile()
    return nc


def kernel(x, W, b, neighbor_idx):
    x = np.asarray(x)
    W = np.asarray(W, dtype=np.float32)
    b = np.asarray(b, dtype=np.float32)
    neighbor_idx = np.asarray(neighbor_idx, dtype=np.int64)
    in_dtype = x.dtype
    x2 = x.reshape(x.shape[0], -1).astype(np.float32)
    N = x2.shape[0]

    plan = _plan(neighbor_idx)
    segs = plan["segs"]
    tb = 0
    for sm in segs:
        sm["tile_base"] = tb
        tb += sm["ntiles"]
        if sm["mode"] == "dense":
            sm["es8"] = 0  # per-core entry start handled via xd slice offset
    NT = tb

    # bf16 table (round-to-nearest-ish)
    xb_bits = ((x2.view(np.uint32) + 0x8000) >> 16).astype(np.uint16)
    xbf = xb_bits.view(_bf16).reshape(N, 8)
    dense_end = max(sm["off"] + plan["levels"][sm["li"]]["T"] for sm in segs if sm["mode"] == "dense")
    dense_rows = min(N, dense_end + 300000)

    per_core = []
    NQS = sum(sm["q"] for sm in segs)
    for c in range(NC):
        gidx, msk, scat, hrows, hout = _core_arrays(plan, neighbor_idx, c)
        # dense phase-B reads xd at per-core offsets: we shift the dense data
        # per core instead (xd differs per core)
        xd = np.zeros((dense_rows, 8), dtype=np.float32)
        for sm in segs:
            if sm["mode"] != "dense":
                continue
            lv = plan["levels"][sm["li"]]
            es = c * sm["chunk"]
            lo = lv["off"] + es
            hi = min(N, lo + P * sm["PL"] + sm["R"] ** 2 + sm["R"] + 2)
            xd[lv["off"]:lv["off"] + (hi - lo)] = x2[lo:hi]
        wt = np.tile((W / 8.0).reshape(1, 64), (P, 1)).astype(np.float32)
        bt = np.tile(b.reshape(1, 8), (P, 1)).astype(np.float32)
        m = dict(xb=np.ascontiguousarray(xbf), xd=xd, gidx=gidx, mskd=msk,
                 wt=wt, bt=bt, scat=scat)
        if plan["HQ"]:
            m["hrows"] = hrows[:plan["HQ"]]
            m["hout"] = hout[:plan["HQ"]]
        per_core.append(m)

    nc = _build_nc(plan, NT, NQS, dense_rows)
    import time as _time
    _t0 = _time.time()
    res = run_bass_kernel_spmd(nc, per_core, list(range(NC)))
    kernel.last_spmd_wall_s = _time.time() - _t0

    full = np.empty((N, 8), dtype=np.float32)
    for c in range(NC):
        co = res.results[c]["out"]
        for sm in segs:
            lv = plan["levels"][sm["li"]]
            es = c * sm["chunk"]
            ecount = min(sm["chunk"], lv["T"] - es)
            if ecount <= 0:
                continue
            full[lv["off"] + es: lv["off"] + es + ecount] = co[sm["o0"]: sm["o0"] + ecount]
    # overlay device-computed fixup rows (host does placement only)
    for c in range(NC):
        fx = res.results[c]["fxo"]
        qpos = 0
        for sm in segs:
            lv = plan["levels"][sm["li"]]
            q = sm["q"]
            fxs = plan["fix"][c][sm["li"]]
            for f, j in enumerate(fxs):
                p, jj = f // q, f % q
                full[lv["off"] + j] = fx[p, (qpos + jj) * 8:(qpos + jj + 1) * 8]
            qpos += q
        if plan["HQ"]:
            nh = plan["HQ"] // P
            for f, ge in enumerate(plan["hard"][c]):
                p, jj = f % P, f // P
                full[ge] = fx[p, (NQS + jj) * 8:(NQS + jj + 1) * 8]
    return full.reshape(x.shape).astype(in_dtype)


# revision 5
# speedup vs baseline: 4549.0357x; 4549.0357x over previous
"""Trainium2 Bass kernel for the multi-level hash-grid context layer.

Algorithm (corner-stream): for the instant-NGP neighbor structure,
neighbor k of entry i equals the corner hash of entry i+s_k for 8 fixed
shifts s_k = {0,1,R,R+1,R^2,R^2+1,R^2+R,R^2+R+1}. So instead of 8 random
gathers per entry we gather ONE corner stream g[j] = x[nbr0[j]] (+ a small
structure-derived extension) and compute the 8-neighbor sum with shifted
streaming adds. Host-side verification compares every entry's actual
neighbor indices against the streamed relation; mismatches (level
boundaries / clamps / arbitrary inputs) are fixed up with direct gathers.

Gather engine: InstDMAGatherAnt (bulk 256B-block gather, int16 indices)
from a bf16 copy of the table; sub-row (16 bf16 rows per 256B block)
extracted on-chip with one-hot masks + reduce. Dense levels (R^3 <= 2^19)
skip the gather entirely (corner stream is the identity there).

All 8 cores run one uniform program (SPMD); each core owns 1/8 of every
level. Work is data-parameterized per core.
"""
import numpy as np
import ml_dtypes

import concourse.bass as bass
import concourse.bacc as bacc
import concourse.mybir as mybir
from concourse.tile import TileContext, add_dep_helper
from concourse.bass_utils import run_bass_kernel_spmd

RES = [16, 20, 25, 32, 40, 51, 64, 81, 102, 128, 161, 203, 256, 323, 406, 512]
CAP = 1 << 19
PRIMES = np.array([1, 2654435761, 805459861], dtype=np.uint32)
NC = 8           # cores
P = 128          # partitions
CT = 64          # gather positions per partition per tile
CB = 192         # phase-B chunk columns (entries per partition per chunk)
BPB = 16         # bf16 rows per 256B gather block
TWO_STREAM_MIN_R = 300

_bf16 = ml_dtypes.bfloat16


def _levels():
    sizes = [min(r ** 3, CAP) for r in RES]
    offs = np.concatenate([[0], np.cumsum(sizes)]).astype(np.int64)
    out = []
    for i, r in enumerate(RES):
        out.append(dict(R=r, T=sizes[i], off=int(offs[i]), dense=r ** 3 <= CAP,
                        chunk=-(-sizes[i] // NC)))
    return out, int(offs[-1])


def _ext_idx(lv, count):
    R = lv["R"]
    j = np.arange(lv["T"], lv["T"] + count, dtype=np.int64)
    cx, cy, cz = (j // (R * R)) % R, (j // R) % R, j % R
    h = (cx.astype(np.uint32) * PRIMES[0]) ^ (cy.astype(np.uint32) * PRIMES[1]) ^ \
        (cz.astype(np.uint32) * PRIMES[2])
    return (lv["off"] + (h % np.uint32(CAP)).astype(np.int64)).astype(np.int64)


def _plan(neighbor_idx):
    levels, N = _levels()
    for lv in levels:
        off, T, R = lv["off"], lv["T"], lv["R"]
        nbr = neighbor_idx[off:off + T]
        E = R * R + R + 2
        g = np.empty(T + E, dtype=np.int64)
        if lv["dense"]:
            g[:] = off + np.arange(T + E, dtype=np.int64)
        else:
            g[:T] = nbr[:, 0]
            g[T:] = _ext_idx(lv, E)
        lv["g_idx"] = g
        ok = np.ones(T, dtype=bool)
        for k, s in enumerate([0, 1, R, R + 1, R * R, R * R + 1, R * R + R, R * R + R + 1]):
            ok &= nbr[:, k] == g[s:s + T]
        # for non-dense levels, the streamed g values must also lie within the
        # level (the bf16 gather window is this level only)
        if not lv["dense"]:
            inlvl = (g >= off) & (g < off + T)
            for s in [0, 1, R, R + 1, R * R, R * R + 1, R * R + R, R * R + R + 1]:
                ok &= inlvl[s:s + T]
        lv["ok"] = ok
        lv["E"] = E

    # per-level segment meta (uniform across cores)
    segs = []
    for li, lv in enumerate(levels):
        PL = -(-lv["chunk"] // P)
        mode = "dense" if lv["dense"] else ("two" if lv["R"] >= TWO_STREAM_MIN_R else "one")
        segs.append(dict(li=li, R=lv["R"], PL=PL, mode=mode,
                         off=lv["off"], T=lv["T"], chunk=lv["chunk"]))

    # fixups per (core, level) and hard fixups (neighbors outside own level)
    fix = [[[] for _ in levels] for _ in range(NC)]
    hard = [[] for _ in range(NC)]
    for li, lv in enumerate(levels):
        off, T = lv["off"], lv["T"]
        bad = np.nonzero(~lv["ok"])[0]
        if len(bad) == 0:
            continue
        nb = neighbor_idx[off + bad]  # [nbad, 8]
        # fixup rows must come from this level's bf16 window (dense: own level
        # f32 path also gathers from the level's bf16 window)
        ok_here = ((nb >= off) & (nb < off + T)).all(axis=1)
        for j, oh in zip(bad, ok_here):
            c = min(int(j // lv["chunk"]), NC - 1)
            if oh:
                fix[c][li].append(int(j))
            else:
                hard[c].append(off + int(j))

    # uniform fixup quotas per level
    fq = []
    for li in range(len(levels)):
        mx = max(len(fix[c][li]) for c in range(NC))
        fq.append(-(-max(mx, 1) // P) * P)
    nhard_max = max(len(h) for h in hard)
    HQ = -(-max(nhard_max, 0) // P) * P  # hard quota (0 if none)

    # gather stream layout per level (positions in "w" space), uniform:
    #   one:   [0, 128*PL + E)            main window
    #   two:   [0, L0) main, [L0, L0+L1) +R^2 window
    #   dense: no main window
    # then fixup region: 8 * fq[li] positions, laid out per-partition:
    #   partition p, slot j, neighbor k -> fixoff + p*(q*8) + j*8 + k
    goff = 0
    for sm in segs:
        R, PL, li = sm["R"], sm["PL"], sm["li"]
        sm["g0"] = goff
        if sm["mode"] == "one":
            sm["len0"] = P * PL + R * R + R + 2
            sm["len1"] = 0
        elif sm["mode"] == "two":
            sm["len0"] = P * PL + R + 2
            sm["len1"] = P * PL + R + 2
        else:
            sm["len0"] = 0
            sm["len1"] = 0
        sm["fixoff"] = sm["len0"] + sm["len1"]
        sm["q"] = fq[li] // P
        slen = sm["fixoff"] + 8 * fq[li]
        slen = -(-slen // (P * CT)) * (P * CT)  # pad to gather-tile multiple
        sm["slen"] = slen
        sm["ntiles"] = slen // (P * CT)
        goff += slen
    GTOT = goff  # gbuf rows per core

    # out layout
    ooff = 0
    for sm in segs:
        sm["o0"] = ooff
        ooff += P * sm["PL"]
    OUT_ROWS = ooff + P  # + dummy tail for fixup padding + hard pads

    return dict(levels=levels, segs=segs, fix=fix, hard=hard, fq=fq, HQ=HQ,
                GTOT=GTOT, OUT_ROWS=OUT_ROWS, N=N)


def _core_arrays(plan, neighbor_idx, c):
    """Build per-core gather idx (int16 blocks), masks (bf16), scatter idx."""
    segs, levels = plan["segs"], plan["levels"]
    rows = np.zeros(plan["GTOT"], dtype=np.int64)   # global row per position
    valid = np.zeros(plan["GTOT"], dtype=bool)
    scat = []
    for sm in segs:
        lv = levels[sm["li"]]
        off, T, R = lv["off"], lv["T"], sm["R"]
        es = c * sm["chunk"]
        g = lv["g_idx"]
        base = sm["g0"]

        def put(dst, start, length):
            s = max(0, min(start, len(g)))
            e = max(0, min(start + length, len(g)))
            if e > s:
                rows[dst + (s - start): dst + (e - start)] = g[s:e]
                valid[dst + (s - start): dst + (e - start)] = True

        if sm["mode"] == "one":
            put(base, es, sm["len0"])
        elif sm["mode"] == "two":
            put(base, es, sm["len0"])
            put(base + sm["len0"], es + R * R, sm["len1"])
        # fixups
        fxs = plan["fix"][c][sm["li"]]
        q = sm["q"]
        soff = np.full((P, q), plan["OUT_ROWS"] - 1, dtype=np.int32)
        for f, j in enumerate(fxs):
            p, jj = f // q, f % q
            w = base + sm["fixoff"] + p * (q * 8) + jj * 8
            rows[w:w + 8] = neighbor_idx[off + j]
            valid[w:w + 8] = True
            soff[p, jj] = sm["o0"] + j
        scat.append(soff)
        # positions of this level must gather within [off, off+T): map invalid
        # to off (block 0)
        lo, hi = base, base + sm["slen"]
        r = rows[lo:hi]
        v = valid[lo:hi]
        r[~v] = off
        np.clip(r, off, off + T - 1, out=r)
        rows[lo:hi] = r

    # per-level: block idx (within level) + sub code, then interleave-feed order
    gidx = np.zeros((plan["GTOT"] // (P * CT), P, CT * 8), dtype=np.int16)
    msk = np.zeros((plan["GTOT"] // (P * CT), P, CT * 16), dtype=_bf16)
    tglob = 0
    for sm in segs:
        lv = levels[sm["li"]]
        lo = sm["g0"]
        GL = sm["slen"] // P
        r = rows[lo:lo + sm["slen"]] - lv["off"]
        blk = (r // BPB).astype(np.int16)
        sub = (r % BPB).astype(np.int16)
        blk_m = blk.reshape(P, GL)
        sub_m = sub.reshape(P, GL)
        for t in range(sm["ntiles"]):
            bt = blk_m[:, t * CT:(t + 1) * CT]          # [P, CT] output-layout
            feed = bt.T.reshape(-1)                     # feed order: i -> (p=i%128, c=i//128)
            w = feed.reshape(CT * 8, 16).T              # wrapped [16, n/16]
            gidx[tglob, :, :] = np.tile(w, (8, 1))
            st = sub_m[:, t * CT:(t + 1) * CT]          # [P, CT]
            m = np.zeros((P, CT, 16), dtype=_bf16)
            np.put_along_axis(m, st[:, :, None].astype(np.int64), _bf16(1.0), axis=2)
            msk[tglob] = m.reshape(P, CT * 16)
            tglob += 1

    scat_all = np.concatenate([s.reshape(P, -1) for s in scat], axis=1)  # [P, sum q]
    # hard fixups
    hq = plan["HQ"]
    hrows = np.zeros((max(hq, 1), 8), dtype=np.int32)
    hout = np.full(max(hq, 1), plan["OUT_ROWS"] - 1, dtype=np.int32)
    segs_by_entry = plan["hard"][c]
    for f, ge in enumerate(segs_by_entry):
        hrows[f] = neighbor_idx[ge]
        # locate out row: level + local j
        for sm in segs:
            lv = levels[sm["li"]]
            if lv["off"] <= ge < lv["off"] + lv["T"]:
                j = ge - lv["off"] - c * sm["chunk"]
                hout[f] = sm["o0"] + j
                break
    return gidx, msk, scat_all.astype(np.int32), hrows, hout


def _build_nc(plan, NT, NQS, dense_rows):
    """Build the uniform Bass program."""
    segs = plan["segs"]
    nc = bacc.Bacc("TRN2", target_bir_lowering=False, debug=False, num_devices=NC)
    f32, bf16, i16, i32 = (mybir.dt.float32, mybir.dt.bfloat16,
                           mybir.dt.int16, mybir.dt.int32)
    N = plan["N"]
    xb = nc.dram_tensor("xb", [N, 8], bf16, kind="ExternalInput")
    xd = nc.dram_tensor("xd", [dense_rows, 8], f32, kind="ExternalInput")
    gidx = nc.dram_tensor("gidx", [NT, P, CT * 8], i16, kind="ExternalInput")
    mskd = nc.dram_tensor("mskd", [NT, P, CT * 16], bf16, kind="ExternalInput")
    wti = nc.dram_tensor("wt", [P, 64], f32, kind="ExternalInput")
    bti = nc.dram_tensor("bt", [P, 8], f32, kind="ExternalInput")
    scat = nc.dram_tensor("scat", [P, NQS], i32, kind="ExternalInput")
    HQ = plan["HQ"]
    if HQ:
        hrowst = nc.dram_tensor("hrows", [HQ, 8], i32, kind="ExternalInput")
        houtt = nc.dram_tensor("hout", [HQ], i32, kind="ExternalInput")
    out = nc.dram_tensor("out", [plan["OUT_ROWS"], 8], f32, kind="ExternalOutput")
    nfx_cols = NQS * 8 + (plan["HQ"] // P) * 8
    fxo = nc.dram_tensor("fxo", [P, max(nfx_cols, 8)], f32, kind="ExternalOutput")
    gbuf = nc.dram_tensor("gbuf", [plan["GTOT"] * 8], f32)

    xbf = xb.ap().rearrange("a b -> (a b)")
    gb = gbuf.ap()
    xdf = xd.ap().rearrange("a b -> (a b)")
    outf = out.ap().rearrange("a b -> (a b)")

    with TileContext(nc) as tc:
        with (
            tc.tile_pool(name="const", bufs=1) as constp,
            tc.tile_pool(name="pa", bufs=2) as pa,
            tc.tile_pool(name="pb", bufs=2) as pb,
            tc.tile_pool(name="pbt", bufs=2) as pbt,
        ):
            wt = constp.tile([P, 64], f32)
            bt = constp.tile([P, 8], f32)
            nc.sync.dma_start(out=wt[:], in_=wti[:])
            nc.sync.dma_start(out=bt[:], in_=bti[:])

            # ---- Phase A: gather + extract per level ----
            for sm in segs:
                if sm["ntiles"] == 0:
                    continue
                lv = plan["levels"][sm["li"]]
                nblk = -(-lv["T"] // BPB)
                win = bass.AP(xb, lv["off"] * 8, [[128, nblk], [1, 128]])
                GL = sm["slen"] // P
                for t in range(sm["ntiles"]):
                    tg = sm["tile_base"] + t
                    idx_sb = pa.tile([P, CT * 8], i16, tag="idx")
                    nc.sync.dma_start(out=idx_sb[:], in_=gidx[tg])
                    mk = pa.tile([P, CT * 16], bf16, tag="msk")
                    nc.sync.dma_start(out=mk[:], in_=mskd[tg])
                    gat = pa.tile([P, CT * 128], bf16, tag="gat")
                    nc.gpsimd.dma_gather(
                        out_ap=gat[:].rearrange("p (c e) -> p c e", e=128),
                        in_ap=win,
                        idxs_ap=idx_sb[:],
                        num_idxs=P * CT,
                        num_idxs_reg=P * CT,
                        elem_size=128,
                        single_packet=False,
                    )
                    tmp = pa.tile([P, CT * 128], bf16, tag="tmp")
                    in0 = gat[:].rearrange("p (c s e) -> p c s e", s=16, e=8)
                    in1 = mk[:].rearrange("p (c s) -> p c s", s=16)
                    in1 = bass.AP(in1.tensor, in1.offset, in1.ap + [[0, 8]])
                    outv = bass.AP(tmp[:].tensor, tmp[:].offset,
                                   [tmp[:].ap[0], [128, CT], [1, 16], [16, 8]])
                    nc.vector.tensor_tensor(out=outv, in0=in0, in1=in1,
                                            op=mybir.AluOpType.mult)
                    rows_t = pa.tile([P, CT * 8], f32, tag="rows")
                    nc.vector.reduce_sum(
                        out=rows_t[:],
                        in_=tmp[:].rearrange("p (ce s) -> p ce s", s=16),
                        axis=mybir.AxisListType.X)
                    dst = bass.AP(gbuf, (sm["g0"] + t * CT) * 8,
                                  [[GL * 8, P], [1, CT * 8]])
                    nc.sync.dma_start(out=dst, in_=rows_t[:])

            # ---- Phase B: streaming shifted sums + affine ----
            out_writes = {}
            for sm in segs:
                R, PL = sm["R"], sm["PL"]
                lv = plan["levels"][sm["li"]]
                if sm["mode"] == "dense":
                    src, sbase = xd, (lv["off"] + 0) * 8
                    # per-core entry start offset added via es
                else:
                    src, sbase = gbuf, sm["g0"] * 8
                nchunk = -(-PL // CB)
                for k in range(nchunk):
                    w = min(CB, PL - k * CB)
                    WN = w + R + 2
                    t0 = pbt.tile([P, WN * 8], f32, tag="t")
                    t1 = pbt.tile([P, WN * 8], f32, tag="t")
                    if sm["mode"] == "two":
                        a0 = bass.AP(src, sbase + k * CB * 8,
                                     [[PL * 8, P], [1, WN * 8]])
                        a1 = bass.AP(src, (sm["g0"] + sm["len0"] + k * CB) * 8,
                                     [[PL * 8, P], [1, WN * 8]])
                    else:
                        a0 = bass.AP(src, sbase + k * CB * 8,
                                     [[PL * 8, P], [1, WN * 8]])
                        a1 = bass.AP(src, sbase + (k * CB + R * R) * 8,
                                     [[PL * 8, P], [1, WN * 8]])
                    nc.sync.dma_start(out=t0[:], in_=a0)
                    nc.sync.dma_start(out=t1[:], in_=a1)
                    # P = t0 + t1 (in place), u = P[j]+P[j+1] (into t1), y = u[j]+u[j+R]
                    nc.vector.tensor_tensor(out=t0[:], in0=t0[:], in1=t1[:],
                                            op=mybir.AluOpType.add)
                    UN = w + R
                    nc.vector.tensor_tensor(out=t1[:, :UN * 8],
                                            in0=t0[:, :UN * 8], in1=t0[:, 8:(UN + 1) * 8],
                                            op=mybir.AluOpType.add)
                    y = pb.tile([P, CB * 8], f32, tag="y")
                    nc.vector.tensor_tensor(out=y[:, :w * 8], in0=t1[:, :w * 8],
                                            in1=t1[:, R * 8:(w + R) * 8],
                                            op=mybir.AluOpType.add)
                    # affine: out = (y/8) @ W'^T + b, W' pre-scaled host-side
                    ybf = pb.tile([P, CB * 8], bf16, tag="ybf")
                    nc.vector.tensor_copy(out=ybf[:, :w * 8], in_=y[:, :w * 8])
                    ot = pb.tile([P, CB * 8], f32, tag="ot")
                    for h in range(2):
                        prod = pb.tile([P, CB * 32], bf16, tag="prod")
                        i0 = ybf[:, :w * 8].rearrange("p (c e) -> p c e", e=8)
                        i0 = bass.AP(i0.tensor, i0.offset,
                                     [i0.ap[0], i0.ap[1], [0, 4], i0.ap[2]])
                        wv = wt[:, h * 32:(h + 1) * 32].rearrange("p (o e) -> p o e", e=8)
                        i1 = bass.AP(wv.tensor, wv.offset,
                                     [wv.ap[0], [0, w], wv.ap[1], wv.ap[2]])
                        pv = prod[:, :w * 32].rearrange("p (c o e) -> p c o e", o=4, e=8)
                        nc.vector.tensor_tensor(out=pv, in0=i0, in1=i1,
                                                op=mybir.AluOpType.mult)
                        nc.vector.reduce_sum(
                            out=bass.AP(ot[:].tensor, ot[:].offset + h * 4,
                                    [ot[:].ap[0], [8, w], [1, 4]]),
                            in_=prod[:, :w * 32].rearrange("p (co e) -> p co e", e=8),
                            axis=mybir.AxisListType.X)
                    bv = bass.AP(bt[:].tensor, bt[:].offset,
                                 [bt[:].ap[0], [0, w], [1, 8]])
                    ov = ot[:, :w * 8].rearrange("p (c e) -> p c e", e=8)
                    nc.vector.tensor_tensor(out=ov, in0=ov, in1=bv,
                                            op=mybir.AluOpType.add)
                    od = bass.AP(out, (sm["o0"] + k * CB) * 8,
                                 [[PL * 8, P], [1, w * 8]])
                    wi = nc.sync.dma_start(out=od, in_=ot[:, :w * 8])
                    out_writes.setdefault(sm["li"], []).append(wi)

            # ---- Phase C: fixups (scatter corrected rows) ----
            qpos = 0
            for sm in segs:
                q = sm["q"]
                if q == 0:
                    continue
                GLb = sm["slen"] // P
                fr = pb.tile([P, q * 64], f32, tag="fr")
                a = bass.AP(gbuf, (sm["g0"] + sm["fixoff"]) * 8,
                            [[q * 64, P], [1, q * 64]])
                nc.sync.dma_start(out=fr[:], in_=a)
                acc = pb.tile([P, q * 8], f32, tag="facc")
                v = fr[:].rearrange("p (j k e) -> p j k e", k=8, e=8)
                nc.vector.tensor_copy(out=acc[:].rearrange("p (j e) -> p j e", e=8),
                                      in_=v[:, :, 0, :])
                for kk in range(1, 8):
                    nc.vector.tensor_tensor(
                        out=acc[:].rearrange("p (j e) -> p j e", e=8),
                        in0=acc[:].rearrange("p (j e) -> p j e", e=8),
                        in1=v[:, :, kk, :], op=mybir.AluOpType.add)
                # affine on acc
                accb = pb.tile([P, q * 8], bf16, tag="faccb")
                nc.vector.tensor_copy(out=accb[:], in_=acc[:])
                facc = pb.tile([P, q * 8], f32, tag="fout")
                for h in range(2):
                    prod = pb.tile([P, q * 32], bf16, tag="fprod")
                    i0 = accb[:].rearrange("p (c e) -> p c e", e=8)
                    i0 = bass.AP(i0.tensor, i0.offset, [i0.ap[0], i0.ap[1], [0, 4], i0.ap[2]])
                    wv = wt[:, h * 32:(h + 1) * 32].rearrange("p (o e) -> p o e", e=8)
                    i1 = bass.AP(wv.tensor, wv.offset, [wv.ap[0], [0, q], wv.ap[1], wv.ap[2]])
                    pv = prod[:].rearrange("p (c o e) -> p c o e", o=4, e=8)
                    nc.vector.tensor_tensor(out=pv, in0=i0, in1=i1, op=mybir.AluOpType.mult)
                    nc.vector.reduce_sum(
                        out=bass.AP(facc[:].tensor, facc[:].offset + h * 4,
                                    [facc[:].ap[0], [8, q], [1, 4]]),
                        in_=prod[:].rearrange("p (co e) -> p co e", e=8),
                        axis=mybir.AxisListType.X)
                bv = bass.AP(bt[:].tensor, bt[:].offset, [bt[:].ap[0], [0, q], [1, 8]])
                fv = facc[:].rearrange("p (c e) -> p c e", e=8)
                nc.vector.tensor_tensor(out=fv, in0=fv, in1=bv, op=mybir.AluOpType.add)
                nc.sync.dma_start(out=fxo[:, qpos * 8:(qpos + q) * 8], in_=facc[:])
                qpos += q

            # ---- hard fixups (rare; neighbors cross levels) ----
            if HQ:
                nh = HQ // P
                hi = pb.tile([P, nh * 8], i32, tag="hi")
                nc.sync.dma_start(out=hi[:], in_=hrowst.ap().rearrange(
                    "(a p) b -> p (a b)", p=P))
                hacc = pb.tile([P, nh * 8], f32, tag="hacc")
                hrow = pb.tile([P, 8], bf16, tag="hrow")
                for j in range(nh):
                    for kk in range(8):
                        nc.gpsimd.indirect_dma_start(
                            out=hrow[:], out_offset=None, in_=xb.ap(),
                            in_offset=bass.IndirectOffsetOnAxis(
                                ap=hi[:, j * 8 + kk:j * 8 + kk + 1], axis=0))
                        if kk == 0:
                            nc.vector.tensor_copy(out=hacc[:, j * 8:(j + 1) * 8], in_=hrow[:])
                        else:
                            nc.vector.tensor_tensor(
                                out=hacc[:, j * 8:(j + 1) * 8],
                                in0=hacc[:, j * 8:(j + 1) * 8], in1=hrow[:],
                                op=mybir.AluOpType.add)
                # affine via 64 MACs would bloat; reuse broadcast trick
                haccb = pb.tile([P, nh * 8], bf16, tag="haccb")
                nc.vector.tensor_copy(out=haccb[:], in_=hacc[:])
                hfin = pb.tile([P, nh * 8], f32, tag="hfin")
                for h in range(2):
                    prod = pb.tile([P, nh * 32], bf16, tag="hprod")
                    i0 = haccb[:].rearrange("p (c e) -> p c e", e=8)
                    i0 = bass.AP(i0.tensor, i0.offset, [i0.ap[0], i0.ap[1], [0, 4], i0.ap[2]])
                    wv = wt[:, h * 32:(h + 1) * 32].rearrange("p (o e) -> p o e", e=8)
                    i1 = bass.AP(wv.tensor, wv.offset, [wv.ap[0], [0, nh], wv.ap[1], wv.ap[2]])
                    pv = prod[:].rearrange("p (c o e) -> p c o e", o=4, e=8)
                    nc.vector.tensor_tensor(out=pv, in0=i0, in1=i1, op=mybir.AluOpType.mult)
                    nc.vector.reduce_sum(
                        out=bass.AP(hfin[:].tensor, hfin[:].offset + h * 4,
                                    [hfin[:].ap[0], [8, nh], [1, 4]]),
                        in_=prod[:].rearrange("p (co e) -> p co e", e=8),
                        axis=mybir.AxisListType.X)
                bv = bass.AP(bt[:].tensor, bt[:].offset, [bt[:].ap[0], [0, nh], [1, 8]])
                hv = hfin[:].rearrange("p (c e) -> p c e", e=8)
                nc.vector.tensor_tensor(out=hv, in0=hv, in1=bv, op=mybir.AluOpType.add)
                nc.sync.dma_start(out=fxo[:, NQS * 8:(NQS + nh) * 8], in_=hfin[:])
    nc.compile()
    return nc


def kernel(x, W, b, neighbor_idx):
    x = np.asarray(x)
    W = np.asarray(W, dtype=np.float32)
    b = np.asarray(b, dtype=np.float32)
    neighbor_idx = np.asarray(neighbor_idx, dtype=np.int64)
    in_dtype = x.dtype
    x2 = x.reshape(x.shape[0], -1).astype(np.float32)
    N = x2.shape[0]

    plan = _plan(neighbor_idx)
    segs = plan["segs"]
    tb = 0
    for sm in segs:
        sm["tile_base"] = tb
        tb += sm["ntiles"]
        if sm["mode"] == "dense":
            sm["es8"] = 0  # per-core entry start handled via xd slice offset
    NT = tb

    # bf16 table (round-to-nearest-ish)
    xb_bits = ((x2.view(np.uint32) + 0x8000) >> 16).astype(np.uint16)
    xbf = xb_bits.view(_bf16).reshape(N, 8)
    dense_end = max(sm["off"] + plan["levels"][sm["li"]]["T"] for sm in segs if sm["mode"] == "dense")
    dense_rows = min(N, dense_end + 300000)

    per_core = []
    NQS = sum(sm["q"] for sm in segs)
    for c in range(NC):
        gidx, msk, scat, hrows, hout = _core_arrays(plan, neighbor_idx, c)
        # dense phase-B reads xd at per-core offsets: we shift the dense data
        # per core instead (xd differs per core)
        xd = np.zeros((dense_rows, 8), dtype=np.float32)
        for sm in segs:
            if sm["mode"] != "dense":
                continue
            lv = plan["levels"][sm["li"]]
            es = c * sm["chunk"]
            lo = lv["off"] + es
            hi = min(N, lo + P * sm["PL"] + sm["R"] ** 2 + sm["R"] + 2)
            xd[lv["off"]:lv["off"] + (hi - lo)] = x2[lo:hi]
        wt = np.tile((W / 8.0).reshape(1, 64), (P, 1)).astype(np.float32)
        bt = np.tile(b.reshape(1, 8), (P, 1)).astype(np.float32)
        m = dict(xb=np.ascontiguousarray(xbf), xd=xd, gidx=gidx, mskd=msk,
                 wt=wt, bt=bt, scat=scat)
        if plan["HQ"]:
            m["hrows"] = hrows[:plan["HQ"]]
            m["hout"] = hout[:plan["HQ"]]
        per_core.append(m)

    nc = _build_nc(plan, NT, NQS, dense_rows)
    kernel.last_nc = nc
    kernel.last_per_core = per_core
    import time as _time
    _t0 = _time.time()
    res = run_bass_kernel_spmd(nc, per_core, list(range(NC)))
    kernel.last_spmd_wall_s = _time.time() - _t0

    full = np.empty((N, 8), dtype=np.float32)
    for c in range(NC):
        co = res.results[c]["out"]
        for sm in segs:
            lv = plan["levels"][sm["li"]]
            es = c * sm["chunk"]
            ecount = min(sm["chunk"], lv["T"] - es)
            if ecount <= 0:
                continue
            full[lv["off"] + es: lv["off"] + es + ecount] = co[sm["o0"]: sm["o0"] + ecount]
    # overlay device-computed fixup rows (host does placement only)
    for c in range(NC):
        fx = res.results[c]["fxo"]
        qpos = 0
        for sm in segs:
            lv = plan["levels"][sm["li"]]
            q = sm["q"]
            fxs = plan["fix"][c][sm["li"]]
            for f, j in enumerate(fxs):
                p, jj = f // q, f % q
                full[lv["off"] + j] = fx[p, (qpos + jj) * 8:(qpos + jj + 1) * 8]
            qpos += q
        if plan["HQ"]:
            nh = plan["HQ"] // P
            for f, ge in enumerate(plan["hard"][c]):
                p, jj = f % P, f // P
                full[ge] = fx[p, (NQS + jj) * 8:(NQS + jj + 1) * 8]
    return full.reshape(x.shape).astype(in_dtype)


# revision 6
# speedup vs baseline: 4936.8593x; 1.0853x over previous
"""Trainium2 Bass kernel for the multi-level hash-grid context layer.

Algorithm (corner-stream): for the instant-NGP neighbor structure,
neighbor k of entry i equals the corner hash of entry i+s_k for 8 fixed
shifts s_k = {0,1,R,R+1,R^2,R^2+1,R^2+R,R^2+R+1}. So instead of 8 random
gathers per entry we gather ONE corner stream g[j] = x[nbr0[j]] (+ a small
structure-derived extension) and compute the 8-neighbor sum with shifted
streaming adds. Host-side verification compares every entry's actual
neighbor indices against the streamed relation; mismatches (level
boundaries / clamps / arbitrary inputs) are fixed up with direct gathers.

Gather engine: InstDMAGatherAnt (bulk 256B-block gather, int16 indices)
from a bf16 copy of the table; sub-row (16 bf16 rows per 256B block)
extracted on-chip with one-hot masks + reduce. Dense levels (R^3 <= 2^19)
skip the gather entirely (corner stream is the identity there).

All 8 cores run one uniform program (SPMD); each core owns 1/8 of every
level. Work is data-parameterized per core.
"""
import numpy as np
import ml_dtypes

import concourse.bass as bass
import concourse.bacc as bacc
import concourse.mybir as mybir
from concourse.tile import TileContext, add_dep_helper
from concourse.bass_utils import run_bass_kernel_spmd

RES = [16, 20, 25, 32, 40, 51, 64, 81, 102, 128, 161, 203, 256, 323, 406, 512]
CAP = 1 << 19
PRIMES = np.array([1, 2654435761, 805459861], dtype=np.uint32)
NC = 8           # cores
P = 128          # partitions
CT = 64          # gather positions per partition per tile
CB = 192         # phase-B chunk columns (entries per partition per chunk)
BPB = 16         # bf16 rows per 256B gather block
TWO_STREAM_MIN_R = 300

_bf16 = ml_dtypes.bfloat16


def _levels():
    sizes = [min(r ** 3, CAP) for r in RES]
    offs = np.concatenate([[0], np.cumsum(sizes)]).astype(np.int64)
    out = []
    for i, r in enumerate(RES):
        out.append(dict(R=r, T=sizes[i], off=int(offs[i]), dense=r ** 3 <= CAP,
                        chunk=-(-sizes[i] // NC)))
    return out, int(offs[-1])


def _ext_idx(lv, count):
    R = lv["R"]
    j = np.arange(lv["T"], lv["T"] + count, dtype=np.int64)
    cx, cy, cz = (j // (R * R)) % R, (j // R) % R, j % R
    h = (cx.astype(np.uint32) * PRIMES[0]) ^ (cy.astype(np.uint32) * PRIMES[1]) ^ \
        (cz.astype(np.uint32) * PRIMES[2])
    return (lv["off"] + (h % np.uint32(CAP)).astype(np.int64)).astype(np.int64)


def _plan(neighbor_idx):
    levels, N = _levels()
    for lv in levels:
        off, T, R = lv["off"], lv["T"], lv["R"]
        nbr = neighbor_idx[off:off + T]
        E = R * R + R + 2
        g = np.empty(T + E, dtype=np.int64)
        if lv["dense"]:
            g[:] = off + np.arange(T + E, dtype=np.int64)
        else:
            g[:T] = nbr[:, 0]
            g[T:] = _ext_idx(lv, E)
        lv["g_idx"] = g
        ok = np.ones(T, dtype=bool)
        for k, s in enumerate([0, 1, R, R + 1, R * R, R * R + 1, R * R + R, R * R + R + 1]):
            ok &= nbr[:, k] == g[s:s + T]
        # for non-dense levels, the streamed g values must also lie within the
        # level (the bf16 gather window is this level only)
        if not lv["dense"]:
            inlvl = (g >= off) & (g < off + T)
            for s in [0, 1, R, R + 1, R * R, R * R + 1, R * R + R, R * R + R + 1]:
                ok &= inlvl[s:s + T]
        lv["ok"] = ok
        lv["E"] = E

    # per-level segment meta (uniform across cores)
    segs = []
    for li, lv in enumerate(levels):
        PL = -(-lv["chunk"] // P)
        mode = "dense" if lv["dense"] else ("two" if lv["R"] >= TWO_STREAM_MIN_R else "one")
        segs.append(dict(li=li, R=lv["R"], PL=PL, mode=mode,
                         off=lv["off"], T=lv["T"], chunk=lv["chunk"]))

    # fixups per (core, level) and hard fixups (neighbors outside own level)
    fix = [[[] for _ in levels] for _ in range(NC)]
    hard = [[] for _ in range(NC)]
    for li, lv in enumerate(levels):
        off, T = lv["off"], lv["T"]
        bad = np.nonzero(~lv["ok"])[0]
        if len(bad) == 0:
            continue
        nb = neighbor_idx[off + bad]  # [nbad, 8]
        # fixup rows must come from this level's bf16 window (dense: own level
        # f32 path also gathers from the level's bf16 window)
        ok_here = ((nb >= off) & (nb < off + T)).all(axis=1)
        for j, oh in zip(bad, ok_here):
            c = min(int(j // lv["chunk"]), NC - 1)
            if oh:
                fix[c][li].append(int(j))
            else:
                hard[c].append(off + int(j))

    # uniform fixup quotas per level
    fq = []
    for li in range(len(levels)):
        mx = max(len(fix[c][li]) for c in range(NC))
        fq.append(-(-max(mx, 1) // P) * P)
    nhard_max = max(len(h) for h in hard)
    HQ = -(-max(nhard_max, 0) // P) * P  # hard quota (0 if none)

    # gather stream layout per level (positions in "w" space), uniform:
    #   one:   [0, 128*PL + E)            main window
    #   two:   [0, L0) main, [L0, L0+L1) +R^2 window
    #   dense: no main window
    # then fixup region: 8 * fq[li] positions, laid out per-partition:
    #   partition p, slot j, neighbor k -> fixoff + p*(q*8) + j*8 + k
    goff = 0
    for sm in segs:
        R, PL, li = sm["R"], sm["PL"], sm["li"]
        sm["g0"] = goff
        if sm["mode"] == "one":
            sm["len0"] = P * PL + R * R + R + 2
            sm["len1"] = 0
        elif sm["mode"] == "two":
            sm["len0"] = P * PL + R + 2
            sm["len1"] = P * PL + R + 2
        else:
            sm["len0"] = 0
            sm["len1"] = 0
        sm["fixoff"] = sm["len0"] + sm["len1"]
        sm["q"] = fq[li] // P
        slen = sm["fixoff"] + 8 * fq[li]
        slen = -(-slen // (P * CT)) * (P * CT)  # pad to gather-tile multiple
        sm["slen"] = slen
        sm["ntiles"] = slen // (P * CT)
        goff += slen
    GTOT = goff  # gbuf rows per core

    # out layout
    ooff = 0
    for sm in segs:
        sm["o0"] = ooff
        ooff += P * sm["PL"]
    OUT_ROWS = ooff + P  # + dummy tail for fixup padding + hard pads

    return dict(levels=levels, segs=segs, fix=fix, hard=hard, fq=fq, HQ=HQ,
                GTOT=GTOT, OUT_ROWS=OUT_ROWS, N=N)


def _core_arrays(plan, neighbor_idx, c):
    """Build per-core gather idx (int16 blocks), masks (bf16), scatter idx."""
    segs, levels = plan["segs"], plan["levels"]
    rows = np.zeros(plan["GTOT"], dtype=np.int64)   # global row per position
    valid = np.zeros(plan["GTOT"], dtype=bool)
    scat = []
    for sm in segs:
        lv = levels[sm["li"]]
        off, T, R = lv["off"], lv["T"], sm["R"]
        es = c * sm["chunk"]
        g = lv["g_idx"]
        base = sm["g0"]

        def put(dst, start, length):
            s = max(0, min(start, len(g)))
            e = max(0, min(start + length, len(g)))
            if e > s:
                rows[dst + (s - start): dst + (e - start)] = g[s:e]
                valid[dst + (s - start): dst + (e - start)] = True

        if sm["mode"] == "one":
            put(base, es, sm["len0"])
        elif sm["mode"] == "two":
            put(base, es, sm["len0"])
            put(base + sm["len0"], es + R * R, sm["len1"])
        # fixups
        fxs = plan["fix"][c][sm["li"]]
        q = sm["q"]
        soff = np.full((P, q), plan["OUT_ROWS"] - 1, dtype=np.int32)
        for f, j in enumerate(fxs):
            p, jj = f // q, f % q
            w = base + sm["fixoff"] + p * (q * 8) + jj * 8
            rows[w:w + 8] = neighbor_idx[off + j]
            valid[w:w + 8] = True
            soff[p, jj] = sm["o0"] + j
        scat.append(soff)
        # positions of this level must gather within [off, off+T): map invalid
        # to off (block 0)
        lo, hi = base, base + sm["slen"]
        r = rows[lo:hi]
        v = valid[lo:hi]
        r[~v] = off
        np.clip(r, off, off + T - 1, out=r)
        rows[lo:hi] = r

    # per-level: block idx (within level) + sub code, then interleave-feed order
    gidx = np.zeros((plan["GTOT"] // (P * CT), P, CT * 8), dtype=np.int16)
    msk = np.zeros((plan["GTOT"] // (P * CT), P, CT * 16), dtype=_bf16)
    tglob = 0
    for sm in segs:
        lv = levels[sm["li"]]
        lo = sm["g0"]
        GL = sm["slen"] // P
        r = rows[lo:lo + sm["slen"]] - lv["off"]
        blk = (r // BPB).astype(np.int16)
        sub = (r % BPB).astype(np.int16)
        blk_m = blk.reshape(P, GL)
        sub_m = sub.reshape(P, GL)
        for t in range(sm["ntiles"]):
            bt = blk_m[:, t * CT:(t + 1) * CT]          # [P, CT] output-layout
            feed = bt.T.reshape(-1)                     # feed order: i -> (p=i%128, c=i//128)
            w = feed.reshape(CT * 8, 16).T              # wrapped [16, n/16]
            gidx[tglob, :, :] = np.tile(w, (8, 1))
            st = sub_m[:, t * CT:(t + 1) * CT]          # [P, CT]
            m = np.zeros((P, CT, 16), dtype=_bf16)
            np.put_along_axis(m, st[:, :, None].astype(np.int64), _bf16(1.0), axis=2)
            msk[tglob] = m.reshape(P, CT * 16)
            tglob += 1

    scat_all = np.concatenate([s.reshape(P, -1) for s in scat], axis=1)  # [P, sum q]
    # hard fixups
    hq = plan["HQ"]
    hrows = np.zeros((max(hq, 1), 8), dtype=np.int32)
    hout = np.full(max(hq, 1), plan["OUT_ROWS"] - 1, dtype=np.int32)
    segs_by_entry = plan["hard"][c]
    for f, ge in enumerate(segs_by_entry):
        hrows[f] = neighbor_idx[ge]
        # locate out row: level + local j
        for sm in segs:
            lv = levels[sm["li"]]
            if lv["off"] <= ge < lv["off"] + lv["T"]:
                j = ge - lv["off"] - c * sm["chunk"]
                hout[f] = sm["o0"] + j
                break
    return gidx, msk, scat_all.astype(np.int32), hrows, hout


def _build_nc(plan, NT, NQS, dense_rows):
    """Build the uniform Bass program."""
    segs = plan["segs"]
    nc = bacc.Bacc("TRN2", target_bir_lowering=False, debug=False, num_devices=NC)
    f32, bf16, i16, i32 = (mybir.dt.float32, mybir.dt.bfloat16,
                           mybir.dt.int16, mybir.dt.int32)
    N = plan["N"]
    xb = nc.dram_tensor("xb", [N, 8], bf16, kind="ExternalInput")
    xd = nc.dram_tensor("xd", [dense_rows, 8], f32, kind="ExternalInput")
    gidx = nc.dram_tensor("gidx", [NT, P, CT * 8], i16, kind="ExternalInput")
    mskd = nc.dram_tensor("mskd", [NT, P, CT * 16], bf16, kind="ExternalInput")
    wti = nc.dram_tensor("wt", [P, 64], f32, kind="ExternalInput")
    bti = nc.dram_tensor("bt", [P, 8], f32, kind="ExternalInput")
    scat = nc.dram_tensor("scat", [P, NQS], i32, kind="ExternalInput")
    HQ = plan["HQ"]
    if HQ:
        hrowst = nc.dram_tensor("hrows", [HQ, 8], i32, kind="ExternalInput")
        houtt = nc.dram_tensor("hout", [HQ], i32, kind="ExternalInput")
    out = nc.dram_tensor("out", [plan["OUT_ROWS"], 8], f32, kind="ExternalOutput")
    nfx_cols = NQS * 8 + (plan["HQ"] // P) * 8
    fxo = nc.dram_tensor("fxo", [P, max(nfx_cols, 8)], f32, kind="ExternalOutput")
    gbuf = nc.dram_tensor("gbuf", [plan["GTOT"] * 8], f32)

    xbf = xb.ap().rearrange("a b -> (a b)")
    gb = gbuf.ap()
    xdf = xd.ap().rearrange("a b -> (a b)")
    outf = out.ap().rearrange("a b -> (a b)")

    with TileContext(nc) as tc:
        with (
            tc.tile_pool(name="const", bufs=1) as constp,
            tc.tile_pool(name="pa", bufs=2) as pa,
            tc.tile_pool(name="pb", bufs=2) as pb,
            tc.tile_pool(name="pbt", bufs=2) as pbt,
        ):
            wt = constp.tile([P, 64], f32)
            bt = constp.tile([P, 8], f32)
            nc.sync.dma_start(out=wt[:], in_=wti[:])
            nc.sync.dma_start(out=bt[:], in_=bti[:])

            # Emission order: dense streaming first (no gather deps), then
            # per hashed level gather -> extract -> stream -> fixups, so the
            # scheduler can overlap level l's streaming with level l+1's
            # gathers instead of serializing whole phases.

            def emit_pa(sm):
                lv = plan["levels"][sm["li"]]
                nblk = -(-lv["T"] // BPB)
                win = bass.AP(xb, lv["off"] * 8, [[128, nblk], [1, 128]])
                GL = sm["slen"] // P
                for t in range(sm["ntiles"]):
                    tg = sm["tile_base"] + t
                    idx_sb = pa.tile([P, CT * 8], i16, tag="idx")
                    nc.scalar.dma_start(out=idx_sb[:], in_=gidx[tg])
                    mk = pa.tile([P, CT * 16], bf16, tag="msk")
                    nc.scalar.dma_start(out=mk[:], in_=mskd[tg])
                    gat = pa.tile([P, CT * 128], bf16, tag="gat")
                    nc.gpsimd.dma_gather(
                        out_ap=gat[:].rearrange("p (c e) -> p c e", e=128),
                        in_ap=win,
                        idxs_ap=idx_sb[:],
                        num_idxs=P * CT,
                        num_idxs_reg=P * CT,
                        elem_size=128,
                        single_packet=False,
                    )
                    tmp = pa.tile([P, CT * 128], bf16, tag="tmp")
                    in0 = gat[:].rearrange("p (c s e) -> p c s e", s=16, e=8)
                    in1 = mk[:].rearrange("p (c s) -> p c s", s=16)
                    in1 = bass.AP(in1.tensor, in1.offset, in1.ap + [[0, 8]])
                    outv = bass.AP(tmp[:].tensor, tmp[:].offset,
                                   [tmp[:].ap[0], [128, CT], [1, 16], [16, 8]])
                    nc.vector.tensor_tensor(out=outv, in0=in0, in1=in1,
                                            op=mybir.AluOpType.mult)
                    rows_t = pa.tile([P, CT * 8], f32, tag="rows")
                    nc.vector.reduce_sum(
                        out=rows_t[:],
                        in_=tmp[:].rearrange("p (ce s) -> p ce s", s=16),
                        axis=mybir.AxisListType.X)
                    dst = bass.AP(gbuf, (sm["g0"] + t * CT) * 8,
                                  [[GL * 8, P], [1, CT * 8]])
                    nc.sync.dma_start(out=dst, in_=rows_t[:])

            def affine_store(srcacc, n, dest_dma):
                """srcacc: f32 [P, n*8] neighbor-sum tile -> (y@W'+b), then
                dest_dma(result_tile, n)."""
                ab = pb.tile([P, n * 8], bf16, tag="ybf")
                nc.vector.tensor_copy(out=ab[:, :n * 8], in_=srcacc)
                ot = pb.tile([P, n * 8], f32, tag="ot")
                for h in range(2):
                    prod = pb.tile([P, n * 32], bf16, tag="prod")
                    i0 = ab[:, :n * 8].rearrange("p (c e) -> p c e", e=8)
                    i0 = bass.AP(i0.tensor, i0.offset,
                                 [i0.ap[0], i0.ap[1], [0, 4], i0.ap[2]])
                    wv = wt[:, h * 32:(h + 1) * 32].rearrange("p (o e) -> p o e", e=8)
                    i1 = bass.AP(wv.tensor, wv.offset,
                                 [wv.ap[0], [0, n], wv.ap[1], wv.ap[2]])
                    pv = prod[:, :n * 32].rearrange("p (c o e) -> p c o e", o=4, e=8)
                    nc.vector.tensor_tensor(out=pv, in0=i0, in1=i1,
                                            op=mybir.AluOpType.mult)
                    nc.vector.reduce_sum(
                        out=bass.AP(ot[:].tensor, ot[:].offset + h * 4,
                                    [ot[:].ap[0], [8, n], [1, 4]]),
                        in_=prod[:, :n * 32].rearrange("p (co e) -> p co e", e=8),
                        axis=mybir.AxisListType.X)
                bv = bass.AP(bt[:].tensor, bt[:].offset, [bt[:].ap[0], [0, n], [1, 8]])
                ov = ot[:, :n * 8].rearrange("p (c e) -> p c e", e=8)
                nc.vector.tensor_tensor(out=ov, in0=ov, in1=bv,
                                        op=mybir.AluOpType.add)
                dest_dma(ot)

            def emit_pb(sm):
                R, PL = sm["R"], sm["PL"]
                lv = plan["levels"][sm["li"]]
                if sm["mode"] == "dense":
                    src, sbase = xd, lv["off"] * 8
                else:
                    src, sbase = gbuf, sm["g0"] * 8
                nchunk = -(-PL // CB)
                for k in range(nchunk):
                    w = min(CB, PL - k * CB)
                    WN = w + R + 2
                    t0 = pbt.tile([P, WN * 8], f32, tag="t")
                    t1 = pbt.tile([P, WN * 8], f32, tag="t")
                    if sm["mode"] == "two":
                        a0 = bass.AP(src, sbase + k * CB * 8,
                                     [[PL * 8, P], [1, WN * 8]])
                        a1 = bass.AP(src, (sm["g0"] + sm["len0"] + k * CB) * 8,
                                     [[PL * 8, P], [1, WN * 8]])
                    else:
                        a0 = bass.AP(src, sbase + k * CB * 8,
                                     [[PL * 8, P], [1, WN * 8]])
                        a1 = bass.AP(src, sbase + (k * CB + R * R) * 8,
                                     [[PL * 8, P], [1, WN * 8]])
                    nc.sync.dma_start(out=t0[:], in_=a0)
                    nc.sync.dma_start(out=t1[:], in_=a1)
                    nc.vector.tensor_tensor(out=t0[:], in0=t0[:], in1=t1[:],
                                            op=mybir.AluOpType.add)
                    UN = w + R
                    nc.vector.tensor_tensor(out=t1[:, :UN * 8],
                                            in0=t0[:, :UN * 8],
                                            in1=t0[:, 8:(UN + 1) * 8],
                                            op=mybir.AluOpType.add)
                    y = pb.tile([P, CB * 8], f32, tag="y")
                    nc.vector.tensor_tensor(out=y[:, :w * 8], in0=t1[:, :w * 8],
                                            in1=t1[:, R * 8:(w + R) * 8],
                                            op=mybir.AluOpType.add)

                    def dest(ot, sm=sm, k=k, w=w, PL=PL):
                        od = bass.AP(out, (sm["o0"] + k * CB) * 8,
                                     [[PL * 8, P], [1, w * 8]])
                        nc.sync.dma_start(out=od, in_=ot[:, :w * 8])
                    affine_store(y[:, :w * 8], w, dest)

            def emit_pc(sm, qpos):
                q = sm["q"]
                if q == 0:
                    return
                fr = pb.tile([P, q * 64], f32, tag="fr")
                a = bass.AP(gbuf, (sm["g0"] + sm["fixoff"]) * 8,
                            [[q * 64, P], [1, q * 64]])
                nc.sync.dma_start(out=fr[:], in_=a)
                acc = pb.tile([P, q * 8], f32, tag="facc")
                v = fr[:].rearrange("p (j k e) -> p j k e", k=8, e=8)
                nc.vector.tensor_copy(out=acc[:].rearrange("p (j e) -> p j e", e=8),
                                      in_=v[:, :, 0, :])
                for kk in range(1, 8):
                    nc.vector.tensor_tensor(
                        out=acc[:].rearrange("p (j e) -> p j e", e=8),
                        in0=acc[:].rearrange("p (j e) -> p j e", e=8),
                        in1=v[:, :, kk, :], op=mybir.AluOpType.add)

                def dest(ot, qpos=qpos, q=q):
                    nc.sync.dma_start(out=fxo[:, qpos * 8:(qpos + q) * 8],
                                      in_=ot[:, :q * 8])
                affine_store(acc[:], q, dest)

            # dense streaming first (independent of all gathers)
            for sm in segs:
                if sm["mode"] == "dense":
                    emit_pb(sm)
            # per-level pipeline
            qpos_map = {}
            qpos = 0
            for sm in segs:
                qpos_map[sm["li"]] = qpos
                qpos += sm["q"]
            for sm in segs:
                if sm["ntiles"]:
                    emit_pa(sm)
                if sm["mode"] != "dense":
                    emit_pb(sm)
                emit_pc(sm, qpos_map[sm["li"]])

            # ---- hard fixups (rare; neighbors cross levels) ----
            if HQ:
                nh = HQ // P
                hi = pb.tile([P, nh * 8], i32, tag="hi")
                nc.sync.dma_start(out=hi[:], in_=hrowst.ap().rearrange(
                    "(a p) b -> p (a b)", p=P))
                hacc = pb.tile([P, nh * 8], f32, tag="hacc")
                hrow = pb.tile([P, 8], bf16, tag="hrow")
                for j in range(nh):
                    for kk in range(8):
                        nc.gpsimd.indirect_dma_start(
                            out=hrow[:], out_offset=None, in_=xb.ap(),
                            in_offset=bass.IndirectOffsetOnAxis(
                                ap=hi[:, j * 8 + kk:j * 8 + kk + 1], axis=0))
                        if kk == 0:
                            nc.vector.tensor_copy(out=hacc[:, j * 8:(j + 1) * 8], in_=hrow[:])
                        else:
                            nc.vector.tensor_tensor(
                                out=hacc[:, j * 8:(j + 1) * 8],
                                in0=hacc[:, j * 8:(j + 1) * 8], in1=hrow[:],
                                op=mybir.AluOpType.add)

                def hdest(ot, nh=nh):
                    nc.sync.dma_start(out=fxo[:, NQS * 8:(NQS + nh) * 8],
                                      in_=ot[:, :nh * 8])
                affine_store(hacc[:], nh, hdest)
    nc.compile()
    return nc


def kernel(x, W, b, neighbor_idx):
    x = np.asarray(x)
    W = np.asarray(W, dtype=np.float32)
    b = np.asarray(b, dtype=np.float32)
    neighbor_idx = np.asarray(neighbor_idx, dtype=np.int64)
    in_dtype = x.dtype
    x2 = x.reshape(x.shape[0], -1).astype(np.float32)
    N = x2.shape[0]

    plan = _plan(neighbor_idx)
    segs = plan["segs"]
    tb = 0
    for sm in segs:
        sm["tile_base"] = tb
        tb += sm["ntiles"]
        if sm["mode"] == "dense":
            sm["es8"] = 0  # per-core entry start handled via xd slice offset
    NT = tb

    # bf16 table (round-to-nearest-ish)
    xb_bits = ((x2.view(np.uint32) + 0x8000) >> 16).astype(np.uint16)
    xbf = xb_bits.view(_bf16).reshape(N, 8)
    dense_end = max(sm["off"] + plan["levels"][sm["li"]]["T"] for sm in segs if sm["mode"] == "dense")
    dense_rows = min(N, dense_end + 300000)

    per_core = []
    NQS = sum(sm["q"] for sm in segs)
    for c in range(NC):
        gidx, msk, scat, hrows, hout = _core_arrays(plan, neighbor_idx, c)
        # dense phase-B reads xd at per-core offsets: we shift the dense data
        # per core instead (xd differs per core)
        xd = np.zeros((dense_rows, 8), dtype=np.float32)
        for sm in segs:
            if sm["mode"] != "dense":
                continue
            lv = plan["levels"][sm["li"]]
            es = c * sm["chunk"]
            lo = lv["off"] + es
            hi = min(N, lo + P * sm["PL"] + sm["R"] ** 2 + sm["R"] + 2)
            xd[lv["off"]:lv["off"] + (hi - lo)] = x2[lo:hi]
        wt = np.tile((W / 8.0).reshape(1, 64), (P, 1)).astype(np.float32)
        bt = np.tile(b.reshape(1, 8), (P, 1)).astype(np.float32)
        m = dict(xb=np.ascontiguousarray(xbf), xd=xd, gidx=gidx, mskd=msk,
                 wt=wt, bt=bt, scat=scat)
        if plan["HQ"]:
            m["hrows"] = hrows[:plan["HQ"]]
            m["hout"] = hout[:plan["HQ"]]
        per_core.append(m)

    nc = _build_nc(plan, NT, NQS, dense_rows)
    kernel.last_nc = nc
    kernel.last_per_core = per_core
    import time as _time
    _t0 = _time.time()
    res = run_bass_kernel_spmd(nc, per_core, list(range(NC)))
    kernel.last_spmd_wall_s = _time.time() - _t0

    full = np.empty((N, 8), dtype=np.float32)
    for c in range(NC):
        co = res.results[c]["out"]
        for sm in segs:
            lv = plan["levels"][sm["li"]]
            es = c * sm["chunk"]
            ecount = min(sm["chunk"], lv["T"] - es)
            if ecount <= 0:
                continue
            full[lv["off"] + es: lv["off"] + es + ecount] = co[sm["o0"]: sm["o0"] + ecount]
    # overlay device-computed fixup rows (host does placement only)
    for c in range(NC):
        fx = res.results[c]["fxo"]
        qpos = 0
        for sm in segs:
            lv = plan["levels"][sm["li"]]
            q = sm["q"]
            fxs = plan["fix"][c][sm["li"]]
            for f, j in enumerate(fxs):
                p, jj = f // q, f % q
                full[lv["off"] + j] = fx[p, (qpos + jj) * 8:(qpos + jj + 1) * 8]
            qpos += q
        if plan["HQ"]:
            nh = plan["HQ"] // P
            for f, ge in enumerate(plan["hard"][c]):
                p, jj = f % P, f // P
                full[ge] = fx[p, (NQS + jj) * 8:(NQS + jj + 1) * 8]
    return full.reshape(x.shape).astype(in_dtype)


# revision 9
# speedup vs baseline: 5056.8114x; 1.0243x over previous
"""Trainium2 Bass kernel for the multi-level hash-grid context layer.

Algorithm (corner-stream): for the instant-NGP neighbor structure,
neighbor k of entry i equals the corner hash of entry i+s_k for 8 fixed
shifts s_k = {0,1,R,R+1,R^2,R^2+1,R^2+R,R^2+R+1}. So instead of 8 random
gathers per entry we gather ONE corner stream g[j] = x[nbr0[j]] (+ a small
structure-derived extension) and compute the 8-neighbor sum with shifted
streaming adds. Host-side verification compares every entry's actual
neighbor indices against the streamed relation; mismatches (level
boundaries / clamps / arbitrary inputs) are fixed up with direct gathers.

Gather engine: InstDMAGatherAnt (bulk 256B-block gather, int16 indices)
from a bf16 copy of the table; sub-row (16 bf16 rows per 256B block)
extracted on-chip with one-hot masks + reduce. Dense levels (R^3 <= 2^19)
skip the gather entirely (corner stream is the identity there).

All 8 cores run one uniform program (SPMD); each core owns 1/8 of every
level. Work is data-parameterized per core.
"""
import numpy as np
import ml_dtypes

import concourse.bass as bass
import concourse.bacc as bacc
import concourse.mybir as mybir
from concourse.tile import TileContext, add_dep_helper
from concourse.bass_utils import run_bass_kernel_spmd

RES = [16, 20, 25, 32, 40, 51, 64, 81, 102, 128, 161, 203, 256, 323, 406, 512]
CAP = 1 << 19
PRIMES = np.array([1, 2654435761, 805459861], dtype=np.uint32)
NC = 8           # cores
P = 128          # partitions
CT = 64          # gather positions per partition per tile
CB = 192         # phase-B chunk columns (entries per partition per chunk)
BPB = 16         # bf16 rows per 256B gather block
TWO_STREAM_MIN_R = 300

_bf16 = ml_dtypes.bfloat16


def _levels():
    sizes = [min(r ** 3, CAP) for r in RES]
    offs = np.concatenate([[0], np.cumsum(sizes)]).astype(np.int64)
    out = []
    for i, r in enumerate(RES):
        out.append(dict(R=r, T=sizes[i], off=int(offs[i]), dense=r ** 3 <= CAP,
                        chunk=-(-sizes[i] // NC)))
    return out, int(offs[-1])


def _ext_idx(lv, count):
    R = lv["R"]
    j = np.arange(lv["T"], lv["T"] + count, dtype=np.int64)
    cx, cy, cz = (j // (R * R)) % R, (j // R) % R, j % R
    h = (cx.astype(np.uint32) * PRIMES[0]) ^ (cy.astype(np.uint32) * PRIMES[1]) ^ \
        (cz.astype(np.uint32) * PRIMES[2])
    return (lv["off"] + (h % np.uint32(CAP)).astype(np.int64)).astype(np.int64)


def _plan(neighbor_idx):
    levels, N = _levels()
    for lv in levels:
        off, T, R = lv["off"], lv["T"], lv["R"]
        nbr = neighbor_idx[off:off + T]
        E = R * R + R + 2
        g = np.empty(T + E, dtype=np.int64)
        if lv["dense"]:
            g[:] = off + np.arange(T + E, dtype=np.int64)
        else:
            g[:T] = nbr[:, 0]
            g[T:] = _ext_idx(lv, E)
        lv["g_idx"] = g
        ok = np.ones(T, dtype=bool)
        for k, s in enumerate([0, 1, R, R + 1, R * R, R * R + 1, R * R + R, R * R + R + 1]):
            ok &= nbr[:, k] == g[s:s + T]
        # for non-dense levels, the streamed g values must also lie within the
        # level (the bf16 gather window is this level only)
        if not lv["dense"]:
            inlvl = (g >= off) & (g < off + T)
            for s in [0, 1, R, R + 1, R * R, R * R + 1, R * R + R, R * R + R + 1]:
                ok &= inlvl[s:s + T]
        lv["ok"] = ok
        lv["E"] = E

    # per-level segment meta (uniform across cores)
    segs = []
    for li, lv in enumerate(levels):
        PL = -(-lv["chunk"] // P)
        mode = "dense" if lv["dense"] else ("two" if lv["R"] >= TWO_STREAM_MIN_R else "one")
        segs.append(dict(li=li, R=lv["R"], PL=PL, mode=mode,
                         off=lv["off"], T=lv["T"], chunk=lv["chunk"]))

    # fixups per (core, level) and hard fixups (neighbors outside own level)
    fix = [[[] for _ in levels] for _ in range(NC)]
    hard = [[] for _ in range(NC)]
    for li, lv in enumerate(levels):
        off, T = lv["off"], lv["T"]
        bad = np.nonzero(~lv["ok"])[0]
        if len(bad) == 0:
            continue
        nb = neighbor_idx[off + bad]  # [nbad, 8]
        # fixup rows must come from this level's bf16 window (dense: own level
        # f32 path also gathers from the level's bf16 window)
        ok_here = ((nb >= off) & (nb < off + T)).all(axis=1)
        for j, oh in zip(bad, ok_here):
            c = min(int(j // lv["chunk"]), NC - 1)
            if oh:
                fix[c][li].append(int(j))
            else:
                hard[c].append(off + int(j))

    # uniform fixup quotas per level
    fq = []
    for li in range(len(levels)):
        mx = max(len(fix[c][li]) for c in range(NC))
        fq.append(-(-max(mx, 1) // P) * P)
    nhard_max = max(len(h) for h in hard)
    HQ = -(-max(nhard_max, 0) // P) * P  # hard quota (0 if none)

    # gather stream layout per level (positions in "w" space), uniform:
    #   one:   [0, 128*PL + E)            main window
    #   two:   [0, L0) main, [L0, L0+L1) +R^2 window
    #   dense: no main window
    # then fixup region: 8 * fq[li] positions, laid out per-partition:
    #   partition p, slot j, neighbor k -> fixoff + p*(q*8) + j*8 + k
    goff = 0
    for sm in segs:
        R, PL, li = sm["R"], sm["PL"], sm["li"]
        sm["g0"] = goff
        if sm["mode"] == "one":
            sm["len0"] = P * PL + R * R + R + 2
            sm["len1"] = 0
        elif sm["mode"] == "two":
            sm["len0"] = P * PL + R + 2
            sm["len1"] = P * PL + R + 2
        else:
            sm["len0"] = 0
            sm["len1"] = 0
        sm["fixoff"] = sm["len0"] + sm["len1"]
        sm["q"] = fq[li] // P
        slen = sm["fixoff"] + 8 * fq[li]
        slen = -(-slen // (P * CT)) * (P * CT)  # pad to gather-tile multiple
        sm["slen"] = slen
        sm["ntiles"] = slen // (P * CT)
        goff += slen
    GTOT = goff  # gbuf rows per core

    # out layout
    ooff = 0
    for sm in segs:
        sm["o0"] = ooff
        ooff += P * sm["PL"]
    OUT_ROWS = ooff + P  # + dummy tail for fixup padding + hard pads

    return dict(levels=levels, segs=segs, fix=fix, hard=hard, fq=fq, HQ=HQ,
                GTOT=GTOT, OUT_ROWS=OUT_ROWS, N=N)


def _core_arrays(plan, neighbor_idx, c):
    """Build per-core gather idx (int16 blocks), masks (bf16), scatter idx."""
    segs, levels = plan["segs"], plan["levels"]
    rows = np.zeros(plan["GTOT"], dtype=np.int64)   # global row per position
    valid = np.zeros(plan["GTOT"], dtype=bool)
    scat = []
    for sm in segs:
        lv = levels[sm["li"]]
        off, T, R = lv["off"], lv["T"], sm["R"]
        es = c * sm["chunk"]
        g = lv["g_idx"]
        base = sm["g0"]

        def put(dst, start, length):
            s = max(0, min(start, len(g)))
            e = max(0, min(start + length, len(g)))
            if e > s:
                rows[dst + (s - start): dst + (e - start)] = g[s:e]
                valid[dst + (s - start): dst + (e - start)] = True

        if sm["mode"] == "one":
            put(base, es, sm["len0"])
        elif sm["mode"] == "two":
            put(base, es, sm["len0"])
            put(base + sm["len0"], es + R * R, sm["len1"])
        # fixups
        fxs = plan["fix"][c][sm["li"]]
        q = sm["q"]
        soff = np.full((P, q), plan["OUT_ROWS"] - 1, dtype=np.int32)
        for f, j in enumerate(fxs):
            p, jj = f // q, f % q
            w = base + sm["fixoff"] + p * (q * 8) + jj * 8
            rows[w:w + 8] = neighbor_idx[off + j]
            valid[w:w + 8] = True
            soff[p, jj] = sm["o0"] + j
        scat.append(soff)
        # positions of this level must gather within [off, off+T): map invalid
        # to off (block 0)
        lo, hi = base, base + sm["slen"]
        r = rows[lo:hi]
        v = valid[lo:hi]
        r[~v] = off
        np.clip(r, off, off + T - 1, out=r)
        rows[lo:hi] = r

    # per-level: block idx (within level) + sub code, then interleave-feed order
    gidx = np.zeros((plan["GTOT"] // (P * CT), P, CT * 8), dtype=np.int16)
    msk = np.zeros((plan["GTOT"] // (P * CT), P, CT * 16), dtype=_bf16)
    tglob = 0
    for sm in segs:
        lv = levels[sm["li"]]
        lo = sm["g0"]
        GL = sm["slen"] // P
        r = rows[lo:lo + sm["slen"]] - lv["off"]
        blk = (r // BPB).astype(np.int16)
        sub = (r % BPB).astype(np.int16)
        blk_m = blk.reshape(P, GL)
        sub_m = sub.reshape(P, GL)
        for t in range(sm["ntiles"]):
            bt = blk_m[:, t * CT:(t + 1) * CT]          # [P, CT] output-layout
            feed = bt.T.reshape(-1)                     # feed order: i -> (p=i%128, c=i//128)
            w = feed.reshape(CT * 8, 16).T              # wrapped [16, n/16]
            gidx[tglob, :, :] = np.tile(w, (8, 1))
            st = sub_m[:, t * CT:(t + 1) * CT]          # [P, CT]
            m = np.zeros((P, CT, 16), dtype=_bf16)
            np.put_along_axis(m, st[:, :, None].astype(np.int64), _bf16(1.0), axis=2)
            msk[tglob] = m.reshape(P, CT * 16)
            tglob += 1

    scat_all = np.concatenate([s.reshape(P, -1) for s in scat], axis=1)  # [P, sum q]
    # hard fixups
    hq = plan["HQ"]
    hrows = np.zeros((max(hq, 1), 8), dtype=np.int32)
    hout = np.full(max(hq, 1), plan["OUT_ROWS"] - 1, dtype=np.int32)
    segs_by_entry = plan["hard"][c]
    for f, ge in enumerate(segs_by_entry):
        hrows[f] = neighbor_idx[ge]
        # locate out row: level + local j
        for sm in segs:
            lv = levels[sm["li"]]
            if lv["off"] <= ge < lv["off"] + lv["T"]:
                j = ge - lv["off"] - c * sm["chunk"]
                hout[f] = sm["o0"] + j
                break
    return gidx, msk, scat_all.astype(np.int32), hrows, hout


def _build_nc(plan, NT, NQS, dense_rows):
    """Build the uniform Bass program."""
    segs = plan["segs"]
    nc = bacc.Bacc("TRN2", target_bir_lowering=False, debug=False, num_devices=NC)
    f32, bf16, i16, i32 = (mybir.dt.float32, mybir.dt.bfloat16,
                           mybir.dt.int16, mybir.dt.int32)
    N = plan["N"]
    xb = nc.dram_tensor("xb", [N, 8], bf16, kind="ExternalInput")
    xd = nc.dram_tensor("xd", [dense_rows, 8], f32, kind="ExternalInput")
    gidx = nc.dram_tensor("gidx", [NT, P, CT * 8], i16, kind="ExternalInput")
    mskd = nc.dram_tensor("mskd", [NT, P, CT * 16], bf16, kind="ExternalInput")
    wti = nc.dram_tensor("wt", [P, 64], f32, kind="ExternalInput")
    bti = nc.dram_tensor("bt", [P, 8], f32, kind="ExternalInput")
    scat = nc.dram_tensor("scat", [P, NQS], i32, kind="ExternalInput")
    HQ = plan["HQ"]
    if HQ:
        hrowst = nc.dram_tensor("hrows", [HQ, 8], i32, kind="ExternalInput")
        houtt = nc.dram_tensor("hout", [HQ], i32, kind="ExternalInput")
    out = nc.dram_tensor("out", [plan["OUT_ROWS"], 8], f32, kind="ExternalOutput")
    nfx_cols = NQS * 8 + (plan["HQ"] // P) * 8
    fxo = nc.dram_tensor("fxo", [P, max(nfx_cols, 8)], f32, kind="ExternalOutput")
    gbuf = nc.dram_tensor("gbuf", [plan["GTOT"] * 8], f32)

    xbf = xb.ap().rearrange("a b -> (a b)")
    gb = gbuf.ap()
    xdf = xd.ap().rearrange("a b -> (a b)")
    outf = out.ap().rearrange("a b -> (a b)")

    with TileContext(nc) as tc:
        with (
            tc.tile_pool(name="const", bufs=1) as constp,
            tc.tile_pool(name="pa", bufs=2) as pa,
            tc.tile_pool(name="pb", bufs=2) as pb,
            tc.tile_pool(name="pbt", bufs=2) as pbt,
        ):
            wt = constp.tile([P, 64], f32)
            bt = constp.tile([P, 8], f32)
            nc.sync.dma_start(out=wt[:], in_=wti[:])
            nc.sync.dma_start(out=bt[:], in_=bti[:])

            # Emission order: dense streaming first (no gather deps), then
            # per hashed level gather -> extract -> stream -> fixups, so the
            # scheduler can overlap level l's streaming with level l+1's
            # gathers instead of serializing whole phases.

            def emit_pa(sm):
                lv = plan["levels"][sm["li"]]
                nblk = -(-lv["T"] // BPB)
                win = bass.AP(xb, lv["off"] * 8, [[128, nblk], [1, 128]])
                GL = sm["slen"] // P
                for t in range(sm["ntiles"]):
                    tg = sm["tile_base"] + t
                    idx_sb = pa.tile([P, CT * 8], i16, tag="idx")
                    nc.scalar.dma_start(out=idx_sb[:], in_=gidx[tg])
                    mk = pa.tile([P, CT * 16], bf16, tag="msk")
                    nc.scalar.dma_start(out=mk[:], in_=mskd[tg])
                    gat = pa.tile([P, CT * 128], bf16, tag="gat")
                    nc.gpsimd.dma_gather(
                        out_ap=gat[:].rearrange("p (c e) -> p c e", e=128),
                        in_ap=win,
                        idxs_ap=idx_sb[:],
                        num_idxs=P * CT,
                        num_idxs_reg=P * CT,
                        elem_size=128,
                        single_packet=False,
                    )
                    tmp = pa.tile([P, CT * 128], bf16, tag="tmp")
                    in0 = gat[:].rearrange("p (c s e) -> p c s e", s=16, e=8)
                    in1 = mk[:].rearrange("p (c s) -> p c s", s=16)
                    in1 = bass.AP(in1.tensor, in1.offset, in1.ap + [[0, 8]])
                    outv = bass.AP(tmp[:].tensor, tmp[:].offset,
                                   [tmp[:].ap[0], [128, CT], [1, 16], [16, 8]])
                    nc.vector.tensor_tensor(out=outv, in0=in0, in1=in1,
                                            op=mybir.AluOpType.mult)
                    rows_t = pa.tile([P, CT * 8], f32, tag="rows")
                    nc.vector.reduce_sum(
                        out=rows_t[:],
                        in_=tmp[:].rearrange("p (ce s) -> p ce s", s=16),
                        axis=mybir.AxisListType.X)
                    dst = bass.AP(gbuf, (sm["g0"] + t * CT) * 8,
                                  [[GL * 8, P], [1, CT * 8]])
                    nc.sync.dma_start(out=dst, in_=rows_t[:])

            def affine_store(srcacc, n, dest_dma):
                """srcacc: f32 [P, n*8] neighbor-sum tile -> (y@W'+b), then
                dest_dma(result_tile, n)."""
                ab = pb.tile([P, n * 8], bf16, tag="ybf")
                nc.vector.tensor_copy(out=ab[:, :n * 8], in_=srcacc)
                ot = pb.tile([P, n * 8], f32, tag="ot")
                for h in range(2):
                    prod = pb.tile([P, n * 32], bf16, tag="prod")
                    i0 = ab[:, :n * 8].rearrange("p (c e) -> p c e", e=8)
                    i0 = bass.AP(i0.tensor, i0.offset,
                                 [i0.ap[0], i0.ap[1], [0, 4], i0.ap[2]])
                    wv = wt[:, h * 32:(h + 1) * 32].rearrange("p (o e) -> p o e", e=8)
                    i1 = bass.AP(wv.tensor, wv.offset,
                                 [wv.ap[0], [0, n], wv.ap[1], wv.ap[2]])
                    pv = prod[:, :n * 32].rearrange("p (c o e) -> p c o e", o=4, e=8)
                    nc.vector.tensor_tensor(out=pv, in0=i0, in1=i1,
                                            op=mybir.AluOpType.mult)
                    nc.vector.reduce_sum(
                        out=bass.AP(ot[:].tensor, ot[:].offset + h * 4,
                                    [ot[:].ap[0], [8, n], [1, 4]]),
                        in_=prod[:, :n * 32].rearrange("p (co e) -> p co e", e=8),
                        axis=mybir.AxisListType.X)
                bv = bass.AP(bt[:].tensor, bt[:].offset, [bt[:].ap[0], [0, n], [1, 8]])
                ov = ot[:, :n * 8].rearrange("p (c e) -> p c e", e=8)
                nc.vector.tensor_tensor(out=ov, in0=ov, in1=bv,
                                        op=mybir.AluOpType.add)
                dest_dma(ot)

            def emit_pb(sm):
                R, PL = sm["R"], sm["PL"]
                lv = plan["levels"][sm["li"]]
                if sm["mode"] == "dense":
                    src, sbase = xd, lv["off"] * 8
                else:
                    src, sbase = gbuf, sm["g0"] * 8
                nchunk = -(-PL // CB)
                for k in range(nchunk):
                    w = min(CB, PL - k * CB)
                    WN = w + R + 2
                    t0 = pbt.tile([P, WN * 8], f32, tag="t")
                    t1 = pbt.tile([P, WN * 8], f32, tag="t")
                    if sm["mode"] == "two":
                        a0 = bass.AP(src, sbase + k * CB * 8,
                                     [[PL * 8, P], [1, WN * 8]])
                        a1 = bass.AP(src, (sm["g0"] + sm["len0"] + k * CB) * 8,
                                     [[PL * 8, P], [1, WN * 8]])
                    else:
                        a0 = bass.AP(src, sbase + k * CB * 8,
                                     [[PL * 8, P], [1, WN * 8]])
                        a1 = bass.AP(src, sbase + (k * CB + R * R) * 8,
                                     [[PL * 8, P], [1, WN * 8]])
                    nc.sync.dma_start(out=t0[:], in_=a0)
                    nc.sync.dma_start(out=t1[:], in_=a1)
                    nc.vector.tensor_tensor(out=t0[:], in0=t0[:], in1=t1[:],
                                            op=mybir.AluOpType.add)
                    UN = w + R
                    nc.vector.tensor_tensor(out=t1[:, :UN * 8],
                                            in0=t0[:, :UN * 8],
                                            in1=t0[:, 8:(UN + 1) * 8],
                                            op=mybir.AluOpType.add)
                    y = pb.tile([P, CB * 8], f32, tag="y")
                    nc.vector.tensor_tensor(out=y[:, :w * 8], in0=t1[:, :w * 8],
                                            in1=t1[:, R * 8:(w + R) * 8],
                                            op=mybir.AluOpType.add)

                    def dest(ot, sm=sm, k=k, w=w, PL=PL):
                        od = bass.AP(out, (sm["o0"] + k * CB) * 8,
                                     [[PL * 8, P], [1, w * 8]])
                        nc.sync.dma_start(out=od, in_=ot[:, :w * 8])
                    affine_store(y[:, :w * 8], w, dest)

            def emit_pc(sm, qpos):
                q = sm["q"]
                if q == 0:
                    return
                fr = pb.tile([P, q * 64], f32, tag="fr")
                a = bass.AP(gbuf, (sm["g0"] + sm["fixoff"]) * 8,
                            [[q * 64, P], [1, q * 64]])
                nc.sync.dma_start(out=fr[:], in_=a)
                acc = pb.tile([P, q * 8], f32, tag="facc")
                v = fr[:].rearrange("p (j k e) -> p j k e", k=8, e=8)
                nc.vector.tensor_copy(out=acc[:].rearrange("p (j e) -> p j e", e=8),
                                      in_=v[:, :, 0, :])
                for kk in range(1, 8):
                    nc.vector.tensor_tensor(
                        out=acc[:].rearrange("p (j e) -> p j e", e=8),
                        in0=acc[:].rearrange("p (j e) -> p j e", e=8),
                        in1=v[:, :, kk, :], op=mybir.AluOpType.add)

                def dest(ot, qpos=qpos, q=q):
                    nc.sync.dma_start(out=fxo[:, qpos * 8:(qpos + q) * 8],
                                      in_=ot[:, :q * 8])
                affine_store(acc[:], q, dest)

            # dense streaming first (independent of all gathers)
            for sm in segs:
                if sm["mode"] == "dense":
                    emit_pb(sm)
            # per-level pipeline
            qpos_map = {}
            qpos = 0
            for sm in segs:
                qpos_map[sm["li"]] = qpos
                qpos += sm["q"]
            for sm in segs:
                if sm["ntiles"]:
                    emit_pa(sm)
                if sm["mode"] != "dense":
                    emit_pb(sm)
                emit_pc(sm, qpos_map[sm["li"]])

            # ---- hard fixups (rare; neighbors cross levels) ----
            if HQ:
                nh = HQ // P
                hi = pb.tile([P, nh * 8], i32, tag="hi")
                nc.sync.dma_start(out=hi[:], in_=hrowst.ap().rearrange(
                    "(a p) b -> p (a b)", p=P))
                hacc = pb.tile([P, nh * 8], f32, tag="hacc")
                hrow = pb.tile([P, 8], bf16, tag="hrow")
                for j in range(nh):
                    for kk in range(8):
                        nc.gpsimd.indirect_dma_start(
                            out=hrow[:], out_offset=None, in_=xb.ap(),
                            in_offset=bass.IndirectOffsetOnAxis(
                                ap=hi[:, j * 8 + kk:j * 8 + kk + 1], axis=0))
                        if kk == 0:
                            nc.vector.tensor_copy(out=hacc[:, j * 8:(j + 1) * 8], in_=hrow[:])
                        else:
                            nc.vector.tensor_tensor(
                                out=hacc[:, j * 8:(j + 1) * 8],
                                in0=hacc[:, j * 8:(j + 1) * 8], in1=hrow[:],
                                op=mybir.AluOpType.add)

                def hdest(ot, nh=nh):
                    nc.sync.dma_start(out=fxo[:, NQS * 8:(NQS + nh) * 8],
                                      in_=ot[:, :nh * 8])
                affine_store(hacc[:], nh, hdest)
    nc.compile()
    return nc


def kernel(x, W, b, neighbor_idx):
    x = np.asarray(x)
    W = np.asarray(W, dtype=np.float32)
    b = np.asarray(b, dtype=np.float32)
    neighbor_idx = np.asarray(neighbor_idx, dtype=np.int64)
    in_dtype = x.dtype
    x2 = x.reshape(x.shape[0], -1).astype(np.float32)
    N = x2.shape[0]

    plan = _plan(neighbor_idx)
    segs = plan["segs"]
    tb = 0
    for sm in segs:
        sm["tile_base"] = tb
        tb += sm["ntiles"]
        if sm["mode"] == "dense":
            sm["es8"] = 0  # per-core entry start handled via xd slice offset
    NT = tb

    # bf16 table (round-to-nearest-ish)
    xb_bits = ((x2.view(np.uint32) + 0x8000) >> 16).astype(np.uint16)
    xbf = xb_bits.view(_bf16).reshape(N, 8)
    dense_end = max(sm["off"] + plan["levels"][sm["li"]]["T"] for sm in segs if sm["mode"] == "dense")
    dense_rows = min(N, dense_end + 300000)

    per_core = []
    NQS = sum(sm["q"] for sm in segs)
    for c in range(NC):
        gidx, msk, scat, hrows, hout = _core_arrays(plan, neighbor_idx, c)
        # dense phase-B reads xd at per-core offsets: we shift the dense data
        # per core instead (xd differs per core)
        xd = np.zeros((dense_rows, 8), dtype=np.float32)
        for sm in segs:
            if sm["mode"] != "dense":
                continue
            lv = plan["levels"][sm["li"]]
            es = c * sm["chunk"]
            lo = lv["off"] + es
            hi = min(N, lo + P * sm["PL"] + sm["R"] ** 2 + sm["R"] + 2)
            xd[lv["off"]:lv["off"] + (hi - lo)] = x2[lo:hi]
        wt = np.tile((W / 8.0).reshape(1, 64), (P, 1)).astype(np.float32)
        bt = np.tile(b.reshape(1, 8), (P, 1)).astype(np.float32)
        m = dict(xb=np.ascontiguousarray(xbf), xd=xd, gidx=gidx, mskd=msk,
                 wt=wt, bt=bt, scat=scat)
        if plan["HQ"]:
            m["hrows"] = hrows[:plan["HQ"]]
            m["hout"] = hout[:plan["HQ"]]
        per_core.append(m)

    nc = _build_nc(plan, NT, NQS, dense_rows)
    kernel.last_nc = nc
    kernel.last_per_core = per_core
    import time as _time
    _t0 = _time.time()
    res = run_bass_kernel_spmd(nc, per_core, list(range(NC)))
    kernel.last_spmd_wall_s = _time.time() - _t0

    full = np.empty((N, 8), dtype=np.float32)
    for c in range(NC):
        co = res.results[c]["out"]
        for sm in segs:
            lv = plan["levels"][sm["li"]]
            es = c * sm["chunk"]
            ecount = min(sm["chunk"], lv["T"] - es)
            if ecount <= 0:
                continue
            full[lv["off"] + es: lv["off"] + es + ecount] = co[sm["o0"]: sm["o0"] + ecount]
    # overlay device-computed fixup rows (host does placement only)
    for c in range(NC):
        fx = res.results[c]["fxo"]
        qpos = 0
        for sm in segs:
            lv = plan["levels"][sm["li"]]
            q = sm["q"]
            fxs = plan["fix"][c][sm["li"]]
            for f, j in enumerate(fxs):
                p, jj = f // q, f % q
                full[lv["off"] + j] = fx[p, (qpos + jj) * 8:(qpos + jj + 1) * 8]
            qpos += q
        if plan["HQ"]:
            nh = plan["HQ"] // P
            for f, ge in enumerate(plan["hard"][c]):
                p, jj = f % P, f // P
                full[ge] = fx[p, (NQS + jj) * 8:(NQS + jj + 1) * 8]
    return full.reshape(x.shape).astype(in_dtype)


# revision 11
# speedup vs baseline: 5850.9538x; 1.1570x over previous
"""Trainium2 Bass kernel for the multi-level hash-grid context layer.

Algorithm (corner-stream): for the instant-NGP neighbor structure,
neighbor k of entry i equals the corner hash of entry i+s_k for 8 fixed
shifts s_k = {0,1,R,R+1,R^2,R^2+1,R^2+R,R^2+R+1}. So instead of 8 random
gathers per entry we gather ONE corner stream g[j] = x[nbr0[j]] (+ a small
structure-derived extension) and compute the 8-neighbor sum with shifted
streaming adds. Host-side verification compares every entry's actual
neighbor indices against the streamed relation; mismatches (level
boundaries / clamps / arbitrary inputs) are fixed up with direct gathers.

Gather engine: InstDMAGatherAnt (bulk 256B-block gather, int16 indices)
from a bf16 copy of the table; sub-row (16 bf16 rows per 256B block)
extracted on-chip with one-hot masks + reduce. Dense levels (R^3 <= 2^19)
skip the gather entirely (corner stream is the identity there).

All 8 cores run one uniform program (SPMD); each core owns 1/8 of every
level. Work is data-parameterized per core.
"""
import numpy as np
import ml_dtypes

import concourse.bass as bass
import concourse.bacc as bacc
import concourse.mybir as mybir
from concourse.tile import TileContext, add_dep_helper
from concourse.bass_utils import run_bass_kernel_spmd

RES = [16, 20, 25, 32, 40, 51, 64, 81, 102, 128, 161, 203, 256, 323, 406, 512]
CAP = 1 << 19
PRIMES = np.array([1, 2654435761, 805459861], dtype=np.uint32)
NC = 8           # cores
P = 128          # partitions
CT = 64          # gather positions per partition per tile
CB = 192         # phase-B chunk columns (entries per partition per chunk)
BPB = 16         # bf16 rows per 256B gather block
TWO_STREAM_MIN_R = 300

_bf16 = ml_dtypes.bfloat16


def _levels():
    sizes = [min(r ** 3, CAP) for r in RES]
    offs = np.concatenate([[0], np.cumsum(sizes)]).astype(np.int64)
    out = []
    for i, r in enumerate(RES):
        out.append(dict(R=r, T=sizes[i], off=int(offs[i]), dense=r ** 3 <= CAP,
                        chunk=-(-sizes[i] // NC)))
    return out, int(offs[-1])


def _ext_idx(lv, count):
    R = lv["R"]
    j = np.arange(lv["T"], lv["T"] + count, dtype=np.int64)
    cx, cy, cz = (j // (R * R)) % R, (j // R) % R, j % R
    h = (cx.astype(np.uint32) * PRIMES[0]) ^ (cy.astype(np.uint32) * PRIMES[1]) ^ \
        (cz.astype(np.uint32) * PRIMES[2])
    return (lv["off"] + (h % np.uint32(CAP)).astype(np.int64)).astype(np.int64)


def _plan(neighbor_idx):
    levels, N = _levels()
    for lv in levels:
        off, T, R = lv["off"], lv["T"], lv["R"]
        nbr = neighbor_idx[off:off + T]
        E = R * R + R + 2
        g = np.empty(T + E, dtype=np.int64)
        if lv["dense"]:
            g[:] = off + np.arange(T + E, dtype=np.int64)
        else:
            g[:T] = nbr[:, 0]
            g[T:] = _ext_idx(lv, E)
        lv["g_idx"] = g
        ok = np.ones(T, dtype=bool)
        for k, s in enumerate([0, 1, R, R + 1, R * R, R * R + 1, R * R + R, R * R + R + 1]):
            ok &= nbr[:, k] == g[s:s + T]
        # for non-dense levels, the streamed g values must also lie within the
        # level (the bf16 gather window is this level only)
        if not lv["dense"]:
            inlvl = (g >= off) & (g < off + T)
            for s in [0, 1, R, R + 1, R * R, R * R + 1, R * R + R, R * R + R + 1]:
                ok &= inlvl[s:s + T]
        lv["ok"] = ok
        lv["E"] = E

    # per-level segment meta (uniform across cores)
    segs = []
    for li, lv in enumerate(levels):
        PL = -(-lv["chunk"] // P)
        mode = "dense" if lv["dense"] else ("two" if lv["R"] >= TWO_STREAM_MIN_R else "one")
        segs.append(dict(li=li, R=lv["R"], PL=PL, mode=mode,
                         off=lv["off"], T=lv["T"], chunk=lv["chunk"]))

    # fixups per (core, level) and hard fixups (neighbors outside own level)
    fix = [[[] for _ in levels] for _ in range(NC)]
    hard = [[] for _ in range(NC)]
    for li, lv in enumerate(levels):
        off, T = lv["off"], lv["T"]
        bad = np.nonzero(~lv["ok"])[0]
        if len(bad) == 0:
            continue
        nb = neighbor_idx[off + bad]  # [nbad, 8]
        # fixup rows must come from this level's bf16 window (dense: own level
        # f32 path also gathers from the level's bf16 window)
        ok_here = ((nb >= off) & (nb < off + T)).all(axis=1)
        for j, oh in zip(bad, ok_here):
            c = min(int(j // lv["chunk"]), NC - 1)
            if oh:
                fix[c][li].append(int(j))
            else:
                hard[c].append(off + int(j))

    # uniform fixup quotas per level
    fq = []
    for li in range(len(levels)):
        mx = max(len(fix[c][li]) for c in range(NC))
        fq.append(-(-max(mx, 1) // P) * P)
    nhard_max = max(len(h) for h in hard)
    HQ = -(-max(nhard_max, 0) // P) * P  # hard quota (0 if none)

    # gather stream layout per level (positions in "w" space), uniform:
    #   one:   [0, 128*PL + E)            main window
    #   two:   [0, L0) main, [L0, L0+L1) +R^2 window
    #   dense: no main window
    # then fixup region: 8 * fq[li] positions, laid out per-partition:
    #   partition p, slot j, neighbor k -> fixoff + p*(q*8) + j*8 + k
    goff = 0
    for sm in segs:
        R, PL, li = sm["R"], sm["PL"], sm["li"]
        sm["g0"] = goff
        if sm["mode"] == "one":
            sm["len0"] = P * PL + R * R + R + 2
            sm["len1"] = 0
        elif sm["mode"] == "two":
            sm["len0"] = P * PL + R + 2
            sm["len1"] = P * PL + R + 2
        else:
            sm["len0"] = 0
            sm["len1"] = 0
        sm["fixoff"] = sm["len0"] + sm["len1"]
        sm["q"] = fq[li] // P
        slen = sm["fixoff"] + 8 * fq[li]
        slen = -(-slen // (P * CT)) * (P * CT)  # pad to gather-tile multiple
        sm["slen"] = slen
        sm["ntiles"] = slen // (P * CT)
        goff += slen
    GTOT = goff  # gbuf rows per core

    # out layout
    ooff = 0
    for sm in segs:
        sm["o0"] = ooff
        ooff += P * sm["PL"]
    OUT_ROWS = ooff + P  # + dummy tail for fixup padding + hard pads

    return dict(levels=levels, segs=segs, fix=fix, hard=hard, fq=fq, HQ=HQ,
                GTOT=GTOT, OUT_ROWS=OUT_ROWS, N=N)


def _core_arrays(plan, neighbor_idx, c):
    """Build per-core gather idx (int16 blocks), masks (bf16), scatter idx."""
    segs, levels = plan["segs"], plan["levels"]
    rows = np.zeros(plan["GTOT"], dtype=np.int64)   # global row per position
    valid = np.zeros(plan["GTOT"], dtype=bool)
    scat = []
    for sm in segs:
        lv = levels[sm["li"]]
        off, T, R = lv["off"], lv["T"], sm["R"]
        es = c * sm["chunk"]
        g = lv["g_idx"]
        base = sm["g0"]

        def put(dst, start, length):
            s = max(0, min(start, len(g)))
            e = max(0, min(start + length, len(g)))
            if e > s:
                rows[dst + (s - start): dst + (e - start)] = g[s:e]
                valid[dst + (s - start): dst + (e - start)] = True

        if sm["mode"] == "one":
            put(base, es, sm["len0"])
        elif sm["mode"] == "two":
            put(base, es, sm["len0"])
            put(base + sm["len0"], es + R * R, sm["len1"])
        # fixups
        fxs = plan["fix"][c][sm["li"]]
        q = sm["q"]
        soff = np.full((P, q), plan["OUT_ROWS"] - 1, dtype=np.int32)
        for f, j in enumerate(fxs):
            p, jj = f // q, f % q
            w = base + sm["fixoff"] + p * (q * 8) + jj * 8
            rows[w:w + 8] = neighbor_idx[off + j]
            valid[w:w + 8] = True
            soff[p, jj] = sm["o0"] + j
        scat.append(soff)
        # positions of this level must gather within [off, off+T): map invalid
        # to off (block 0)
        lo, hi = base, base + sm["slen"]
        r = rows[lo:hi]
        v = valid[lo:hi]
        r[~v] = off
        np.clip(r, off, off + T - 1, out=r)
        rows[lo:hi] = r

    # per-level: block idx (within level) + sub code, then interleave-feed order
    gidx = np.zeros((plan["GTOT"] // (P * CT), P, CT * 8), dtype=np.int16)
    msk = np.zeros((plan["GTOT"] // (P * CT), P, CT * 16), dtype=_bf16)
    tglob = 0
    for sm in segs:
        lv = levels[sm["li"]]
        lo = sm["g0"]
        GL = sm["slen"] // P
        r = rows[lo:lo + sm["slen"]] - lv["off"]
        blk = (r // BPB).astype(np.int16)
        sub = (r % BPB).astype(np.int16)
        blk_m = blk.reshape(P, GL)
        sub_m = sub.reshape(P, GL)
        for t in range(sm["ntiles"]):
            bt = blk_m[:, t * CT:(t + 1) * CT]          # [P, CT] output-layout
            feed = bt.T.reshape(-1)                     # feed order: i -> (p=i%128, c=i//128)
            w = feed.reshape(CT * 8, 16).T              # wrapped [16, n/16]
            gidx[tglob, :, :] = np.tile(w, (8, 1))
            st = sub_m[:, t * CT:(t + 1) * CT]          # [P, CT]
            m = np.zeros((P, CT, 16), dtype=_bf16)
            np.put_along_axis(m, st[:, :, None].astype(np.int64), _bf16(1.0), axis=2)
            msk[tglob] = m.reshape(P, CT * 16)
            tglob += 1

    scat_all = np.concatenate([s.reshape(P, -1) for s in scat], axis=1)  # [P, sum q]
    # hard fixups
    hq = plan["HQ"]
    hrows = np.zeros((max(hq, 1), 8), dtype=np.int32)
    hout = np.full(max(hq, 1), plan["OUT_ROWS"] - 1, dtype=np.int32)
    segs_by_entry = plan["hard"][c]
    for f, ge in enumerate(segs_by_entry):
        hrows[f] = neighbor_idx[ge]
        # locate out row: level + local j
        for sm in segs:
            lv = levels[sm["li"]]
            if lv["off"] <= ge < lv["off"] + lv["T"]:
                j = ge - lv["off"] - c * sm["chunk"]
                hout[f] = sm["o0"] + j
                break
    return gidx, msk, scat_all.astype(np.int32), hrows, hout


def _build_nc(plan, NT, NQS, dense_rows):
    """Build the uniform Bass program."""
    segs = plan["segs"]
    nc = bacc.Bacc("TRN2", target_bir_lowering=False, debug=False, num_devices=NC)
    f32, bf16, i16, i32 = (mybir.dt.float32, mybir.dt.bfloat16,
                           mybir.dt.int16, mybir.dt.int32)
    N = plan["N"]
    xb = nc.dram_tensor("xb", [N, 8], bf16, kind="ExternalInput")
    xd = nc.dram_tensor("xd", [dense_rows, 8], f32, kind="ExternalInput")
    gidx = nc.dram_tensor("gidx", [NT, P, CT * 8], i16, kind="ExternalInput")
    mskd = nc.dram_tensor("mskd", [NT, P, CT * 16], bf16, kind="ExternalInput")
    wti = nc.dram_tensor("wt", [P, 64], f32, kind="ExternalInput")
    bti = nc.dram_tensor("bt", [P, 8], f32, kind="ExternalInput")
    scat = nc.dram_tensor("scat", [P, NQS], i32, kind="ExternalInput")
    HQ = plan["HQ"]
    if HQ:
        hrowst = nc.dram_tensor("hrows", [HQ, 8], i32, kind="ExternalInput")
        houtt = nc.dram_tensor("hout", [HQ], i32, kind="ExternalInput")
    out = nc.dram_tensor("out", [plan["OUT_ROWS"], 8], f32, kind="ExternalOutput")
    nfx_cols = NQS * 8 + (plan["HQ"] // P) * 8
    fxo = nc.dram_tensor("fxo", [P, max(nfx_cols, 8)], f32, kind="ExternalOutput")
    gbuf = nc.dram_tensor("gbuf", [plan["GTOT"] * 8], f32)

    xbf = xb.ap().rearrange("a b -> (a b)")
    gb = gbuf.ap()
    xdf = xd.ap().rearrange("a b -> (a b)")
    outf = out.ap().rearrange("a b -> (a b)")

    with TileContext(nc) as tc:
        with (
            tc.tile_pool(name="const", bufs=1) as constp,
            tc.tile_pool(name="pa", bufs=2) as pa,
            tc.tile_pool(name="pasc", bufs=1) as pasc,
            tc.tile_pool(name="pb", bufs=2) as pb,
            tc.tile_pool(name="pbt", bufs=2) as pbt,
        ):
            wt = constp.tile([P, 64], f32)
            bt = constp.tile([P, 8], f32)
            nc.sync.dma_start(out=wt[:], in_=wti[:])
            nc.sync.dma_start(out=bt[:], in_=bti[:])

            # Emission order: dense streaming first (no gather deps), then
            # per hashed level gather -> extract -> stream -> fixups, so the
            # scheduler can overlap level l's streaming with level l+1's
            # gathers instead of serializing whole phases.

            def emit_pa(sm):
                lv = plan["levels"][sm["li"]]
                nblk = -(-lv["T"] // BPB)
                win = bass.AP(xb, lv["off"] * 8, [[128, nblk], [1, 128]])
                GL = sm["slen"] // P
                for t in range(sm["ntiles"]):
                    tg = sm["tile_base"] + t
                    idx_sb = pa.tile([P, CT * 8], i16, tag="idx")
                    nc.scalar.dma_start(out=idx_sb[:], in_=gidx[tg])
                    mk = pa.tile([P, CT * 16], bf16, tag="msk")
                    nc.scalar.dma_start(out=mk[:], in_=mskd[tg])
                    gat = pa.tile([P, CT * 128], bf16, tag="gat")
                    nc.gpsimd.dma_gather(
                        out_ap=gat[:].rearrange("p (c e) -> p c e", e=128),
                        in_ap=win,
                        idxs_ap=idx_sb[:],
                        num_idxs=P * CT,
                        num_idxs_reg=P * CT,
                        elem_size=128,
                        single_packet=False,
                    )
                    # one-hot mask multiply with a fully contiguous write,
                    # then a contiguous tree-reduction over the 16 sub-rows
                    # (masks are one-hot, so bf16 partials are exact).
                    tmp = pa.tile([P, CT * 128], bf16, tag="tmp")
                    in0 = gat[:].rearrange("p (c s e) -> p c s e", s=16, e=8)
                    in1 = mk[:].rearrange("p (c s) -> p c s", s=16)
                    in1 = bass.AP(in1.tensor, in1.offset, in1.ap + [[0, 8]])
                    outv = tmp[:].rearrange("p (c s e) -> p c s e", s=16, e=8)
                    nc.vector.tensor_tensor(out=outv, in0=in0, in1=in1,
                                            op=mybir.AluOpType.mult)
                    a1 = pasc.tile([P, CT * 64], bf16, tag="a1")
                    nc.vector.tensor_tensor(
                        out=a1[:],
                        in0=bass.AP(tmp[:].tensor, tmp[:].offset,
                                    [tmp[:].ap[0], [128, CT], [1, 64]]),
                        in1=bass.AP(tmp[:].tensor, tmp[:].offset + 64,
                                    [tmp[:].ap[0], [128, CT], [1, 64]]),
                        op=mybir.AluOpType.add)
                    a2 = pasc.tile([P, CT * 32], bf16, tag="a2")
                    nc.vector.tensor_tensor(
                        out=a2[:],
                        in0=bass.AP(a1[:].tensor, a1[:].offset,
                                    [a1[:].ap[0], [64, CT], [1, 32]]),
                        in1=bass.AP(a1[:].tensor, a1[:].offset + 32,
                                    [a1[:].ap[0], [64, CT], [1, 32]]),
                        op=mybir.AluOpType.add)
                    a3 = pasc.tile([P, CT * 16], bf16, tag="a1")
                    nc.vector.tensor_tensor(
                        out=a3[:],
                        in0=bass.AP(a2[:].tensor, a2[:].offset,
                                    [a2[:].ap[0], [32, CT], [1, 16]]),
                        in1=bass.AP(a2[:].tensor, a2[:].offset + 16,
                                    [a2[:].ap[0], [32, CT], [1, 16]]),
                        op=mybir.AluOpType.add)
                    rows_t = pa.tile([P, CT * 8], f32, tag="rows")
                    nc.vector.tensor_tensor(
                        out=rows_t[:],
                        in0=bass.AP(a3[:].tensor, a3[:].offset,
                                    [a3[:].ap[0], [16, CT], [1, 8]]),
                        in1=bass.AP(a3[:].tensor, a3[:].offset + 8,
                                    [a3[:].ap[0], [16, CT], [1, 8]]),
                        op=mybir.AluOpType.add)
                    dst = bass.AP(gbuf, (sm["g0"] + t * CT) * 8,
                                  [[GL * 8, P], [1, CT * 8]])
                    nc.sync.dma_start(out=dst, in_=rows_t[:])

            def affine_store(srcacc, n, dest_dma):
                """srcacc: f32 [P, n*8] neighbor-sum tile -> (y@W'+b), then
                dest_dma(result_tile, n)."""
                ab = pb.tile([P, n * 8], bf16, tag="ybf")
                nc.vector.tensor_copy(out=ab[:, :n * 8], in_=srcacc)
                ot = pb.tile([P, n * 8], f32, tag="ot")
                for h in range(2):
                    prod = pb.tile([P, n * 32], bf16, tag="prod")
                    i0 = ab[:, :n * 8].rearrange("p (c e) -> p c e", e=8)
                    i0 = bass.AP(i0.tensor, i0.offset,
                                 [i0.ap[0], i0.ap[1], [0, 4], i0.ap[2]])
                    wv = wt[:, h * 32:(h + 1) * 32].rearrange("p (o e) -> p o e", e=8)
                    i1 = bass.AP(wv.tensor, wv.offset,
                                 [wv.ap[0], [0, n], wv.ap[1], wv.ap[2]])
                    pv = prod[:, :n * 32].rearrange("p (c o e) -> p c o e", o=4, e=8)
                    nc.vector.tensor_tensor(out=pv, in0=i0, in1=i1,
                                            op=mybir.AluOpType.mult)
                    nc.vector.reduce_sum(
                        out=bass.AP(ot[:].tensor, ot[:].offset + h * 4,
                                    [ot[:].ap[0], [8, n], [1, 4]]),
                        in_=prod[:, :n * 32].rearrange("p (co e) -> p co e", e=8),
                        axis=mybir.AxisListType.X)
                bv = bass.AP(bt[:].tensor, bt[:].offset, [bt[:].ap[0], [0, n], [1, 8]])
                ov = ot[:, :n * 8].rearrange("p (c e) -> p c e", e=8)
                nc.vector.tensor_tensor(out=ov, in0=ov, in1=bv,
                                        op=mybir.AluOpType.add)
                dest_dma(ot)

            def emit_pb(sm):
                R, PL = sm["R"], sm["PL"]
                lv = plan["levels"][sm["li"]]
                if sm["mode"] == "dense":
                    src, sbase = xd, lv["off"] * 8
                else:
                    src, sbase = gbuf, sm["g0"] * 8
                nchunk = -(-PL // CB)
                for k in range(nchunk):
                    w = min(CB, PL - k * CB)
                    WN = w + R + 2
                    t0 = pbt.tile([P, WN * 8], f32, tag="t")
                    t1 = pbt.tile([P, WN * 8], f32, tag="t")
                    if sm["mode"] == "two":
                        a0 = bass.AP(src, sbase + k * CB * 8,
                                     [[PL * 8, P], [1, WN * 8]])
                        a1 = bass.AP(src, (sm["g0"] + sm["len0"] + k * CB) * 8,
                                     [[PL * 8, P], [1, WN * 8]])
                    else:
                        a0 = bass.AP(src, sbase + k * CB * 8,
                                     [[PL * 8, P], [1, WN * 8]])
                        a1 = bass.AP(src, sbase + (k * CB + R * R) * 8,
                                     [[PL * 8, P], [1, WN * 8]])
                    nc.sync.dma_start(out=t0[:], in_=a0)
                    nc.sync.dma_start(out=t1[:], in_=a1)
                    nc.vector.tensor_tensor(out=t0[:], in0=t0[:], in1=t1[:],
                                            op=mybir.AluOpType.add)
                    UN = w + R
                    nc.vector.tensor_tensor(out=t1[:, :UN * 8],
                                            in0=t0[:, :UN * 8],
                                            in1=t0[:, 8:(UN + 1) * 8],
                                            op=mybir.AluOpType.add)
                    y = pb.tile([P, CB * 8], f32, tag="y")
                    nc.vector.tensor_tensor(out=y[:, :w * 8], in0=t1[:, :w * 8],
                                            in1=t1[:, R * 8:(w + R) * 8],
                                            op=mybir.AluOpType.add)

                    def dest(ot, sm=sm, k=k, w=w, PL=PL):
                        od = bass.AP(out, (sm["o0"] + k * CB) * 8,
                                     [[PL * 8, P], [1, w * 8]])
                        nc.sync.dma_start(out=od, in_=ot[:, :w * 8])
                    affine_store(y[:, :w * 8], w, dest)

            def emit_pc(sm, qpos):
                q = sm["q"]
                if q == 0:
                    return
                fr = pb.tile([P, q * 64], f32, tag="fr")
                a = bass.AP(gbuf, (sm["g0"] + sm["fixoff"]) * 8,
                            [[q * 64, P], [1, q * 64]])
                nc.sync.dma_start(out=fr[:], in_=a)
                acc = pb.tile([P, q * 8], f32, tag="facc")
                v = fr[:].rearrange("p (j k e) -> p j k e", k=8, e=8)
                nc.vector.tensor_copy(out=acc[:].rearrange("p (j e) -> p j e", e=8),
                                      in_=v[:, :, 0, :])
                for kk in range(1, 8):
                    nc.vector.tensor_tensor(
                        out=acc[:].rearrange("p (j e) -> p j e", e=8),
                        in0=acc[:].rearrange("p (j e) -> p j e", e=8),
                        in1=v[:, :, kk, :], op=mybir.AluOpType.add)

                def dest(ot, qpos=qpos, q=q):
                    nc.sync.dma_start(out=fxo[:, qpos * 8:(qpos + q) * 8],
                                      in_=ot[:, :q * 8])
                affine_store(acc[:], q, dest)

            # dense streaming first (independent of all gathers)
            for sm in segs:
                if sm["mode"] == "dense":
                    emit_pb(sm)
            # per-level pipeline
            qpos_map = {}
            qpos = 0
            for sm in segs:
                qpos_map[sm["li"]] = qpos
                qpos += sm["q"]
            for sm in segs:
                if sm["ntiles"]:
                    emit_pa(sm)
                if sm["mode"] != "dense":
                    emit_pb(sm)
                emit_pc(sm, qpos_map[sm["li"]])

            # ---- hard fixups (rare; neighbors cross levels) ----
            if HQ:
                nh = HQ // P
                hi = pb.tile([P, nh * 8], i32, tag="hi")
                nc.sync.dma_start(out=hi[:], in_=hrowst.ap().rearrange(
                    "(a p) b -> p (a b)", p=P))
                hacc = pb.tile([P, nh * 8], f32, tag="hacc")
                hrow = pb.tile([P, 8], bf16, tag="hrow")
                for j in range(nh):
                    for kk in range(8):
                        nc.gpsimd.indirect_dma_start(
                            out=hrow[:], out_offset=None, in_=xb.ap(),
                            in_offset=bass.IndirectOffsetOnAxis(
                                ap=hi[:, j * 8 + kk:j * 8 + kk + 1], axis=0))
                        if kk == 0:
                            nc.vector.tensor_copy(out=hacc[:, j * 8:(j + 1) * 8], in_=hrow[:])
                        else:
                            nc.vector.tensor_tensor(
                                out=hacc[:, j * 8:(j + 1) * 8],
                                in0=hacc[:, j * 8:(j + 1) * 8], in1=hrow[:],
                                op=mybir.AluOpType.add)

                def hdest(ot, nh=nh):
                    nc.sync.dma_start(out=fxo[:, NQS * 8:(NQS + nh) * 8],
                                      in_=ot[:, :nh * 8])
                affine_store(hacc[:], nh, hdest)
    nc.compile()
    return nc


def kernel(x, W, b, neighbor_idx):
    x = np.asarray(x)
    W = np.asarray(W, dtype=np.float32)
    b = np.asarray(b, dtype=np.float32)
    neighbor_idx = np.asarray(neighbor_idx, dtype=np.int64)
    in_dtype = x.dtype
    x2 = x.reshape(x.shape[0], -1).astype(np.float32)
    N = x2.shape[0]

    plan = _plan(neighbor_idx)
    segs = plan["segs"]
    tb = 0
    for sm in segs:
        sm["tile_base"] = tb
        tb += sm["ntiles"]
        if sm["mode"] == "dense":
            sm["es8"] = 0  # per-core entry start handled via xd slice offset
    NT = tb

    # bf16 table (round-to-nearest-ish)
    xb_bits = ((x2.view(np.uint32) + 0x8000) >> 16).astype(np.uint16)
    xbf = xb_bits.view(_bf16).reshape(N, 8)
    dense_end = max(sm["off"] + plan["levels"][sm["li"]]["T"] for sm in segs if sm["mode"] == "dense")
    dense_rows = min(N, dense_end + 300000)

    per_core = []
    NQS = sum(sm["q"] for sm in segs)
    for c in range(NC):
        gidx, msk, scat, hrows, hout = _core_arrays(plan, neighbor_idx, c)
        # dense phase-B reads xd at per-core offsets: we shift the dense data
        # per core instead (xd differs per core)
        xd = np.zeros((dense_rows, 8), dtype=np.float32)
        for sm in segs:
            if sm["mode"] != "dense":
                continue
            lv = plan["levels"][sm["li"]]
            es = c * sm["chunk"]
            lo = lv["off"] + es
            hi = min(N, lo + P * sm["PL"] + sm["R"] ** 2 + sm["R"] + 2)
            xd[lv["off"]:lv["off"] + (hi - lo)] = x2[lo:hi]
        wt = np.tile((W / 8.0).reshape(1, 64), (P, 1)).astype(np.float32)
        bt = np.tile(b.reshape(1, 8), (P, 1)).astype(np.float32)
        m = dict(xb=np.ascontiguousarray(xbf), xd=xd, gidx=gidx, mskd=msk,
                 wt=wt, bt=bt, scat=scat)
        if plan["HQ"]:
            m["hrows"] = hrows[:plan["HQ"]]
            m["hout"] = hout[:plan["HQ"]]
        per_core.append(m)

    nc = _build_nc(plan, NT, NQS, dense_rows)
    kernel.last_nc = nc
    kernel.last_per_core = per_core
    import time as _time
    _t0 = _time.time()
    res = run_bass_kernel_spmd(nc, per_core, list(range(NC)))
    kernel.last_spmd_wall_s = _time.time() - _t0

    full = np.empty((N, 8), dtype=np.float32)
    for c in range(NC):
        co = res.results[c]["out"]
        for sm in segs:
            lv = plan["levels"][sm["li"]]
            es = c * sm["chunk"]
            ecount = min(sm["chunk"], lv["T"] - es)
            if ecount <= 0:
                continue
            full[lv["off"] + es: lv["off"] + es + ecount] = co[sm["o0"]: sm["o0"] + ecount]
    # overlay device-computed fixup rows (host does placement only)
    for c in range(NC):
        fx = res.results[c]["fxo"]
        qpos = 0
        for sm in segs:
            lv = plan["levels"][sm["li"]]
            q = sm["q"]
            fxs = plan["fix"][c][sm["li"]]
            for f, j in enumerate(fxs):
                p, jj = f // q, f % q
                full[lv["off"] + j] = fx[p, (qpos + jj) * 8:(qpos + jj + 1) * 8]
            qpos += q
        if plan["HQ"]:
            nh = plan["HQ"] // P
            for f, ge in enumerate(plan["hard"][c]):
                p, jj = f % P, f // P
                full[ge] = fx[p, (NQS + jj) * 8:(NQS + jj + 1) * 8]
    return full.reshape(x.shape).astype(in_dtype)


# revision 12
# speedup vs baseline: 5862.9293x; 1.0020x over previous
"""Trainium2 Bass kernel for the multi-level hash-grid context layer.

Algorithm (corner-stream): for the instant-NGP neighbor structure,
neighbor k of entry i equals the corner hash of entry i+s_k for 8 fixed
shifts s_k = {0,1,R,R+1,R^2,R^2+1,R^2+R,R^2+R+1}. So instead of 8 random
gathers per entry we gather ONE corner stream g[j] = x[nbr0[j]] (+ a small
structure-derived extension) and compute the 8-neighbor sum with shifted
streaming adds. Host-side verification compares every entry's actual
neighbor indices against the streamed relation; mismatches (level
boundaries / clamps / arbitrary inputs) are fixed up with direct gathers.

Gather engine: InstDMAGatherAnt (bulk 256B-block gather, int16 indices)
from a bf16 copy of the table; sub-row (16 bf16 rows per 256B block)
extracted on-chip with one-hot masks + reduce. Dense levels (R^3 <= 2^19)
skip the gather entirely (corner stream is the identity there).

All 8 cores run one uniform program (SPMD); each core owns 1/8 of every
level. Work is data-parameterized per core.
"""
import numpy as np
import ml_dtypes

import concourse.bass as bass
import concourse.bacc as bacc
import concourse.mybir as mybir
from concourse.tile import TileContext, add_dep_helper
from concourse.bass_utils import run_bass_kernel_spmd

RES = [16, 20, 25, 32, 40, 51, 64, 81, 102, 128, 161, 203, 256, 323, 406, 512]
CAP = 1 << 19
PRIMES = np.array([1, 2654435761, 805459861], dtype=np.uint32)
NC = 8           # cores
P = 128          # partitions
CT = 64          # gather positions per partition per tile
CB = 192         # phase-B chunk columns (entries per partition per chunk)
BPB = 16         # bf16 rows per 256B gather block
TWO_STREAM_MIN_R = 300

_bf16 = ml_dtypes.bfloat16


def _levels():
    sizes = [min(r ** 3, CAP) for r in RES]
    offs = np.concatenate([[0], np.cumsum(sizes)]).astype(np.int64)
    out = []
    for i, r in enumerate(RES):
        out.append(dict(R=r, T=sizes[i], off=int(offs[i]), dense=r ** 3 <= CAP,
                        chunk=-(-sizes[i] // NC)))
    return out, int(offs[-1])


def _ext_idx(lv, count):
    R = lv["R"]
    j = np.arange(lv["T"], lv["T"] + count, dtype=np.int64)
    cx, cy, cz = (j // (R * R)) % R, (j // R) % R, j % R
    h = (cx.astype(np.uint32) * PRIMES[0]) ^ (cy.astype(np.uint32) * PRIMES[1]) ^ \
        (cz.astype(np.uint32) * PRIMES[2])
    return (lv["off"] + (h % np.uint32(CAP)).astype(np.int64)).astype(np.int64)


def _plan(neighbor_idx):
    levels, N = _levels()
    for lv in levels:
        off, T, R = lv["off"], lv["T"], lv["R"]
        nbr = neighbor_idx[off:off + T]
        E = R * R + R + 2
        g = np.empty(T + E, dtype=np.int64)
        if lv["dense"]:
            g[:] = off + np.arange(T + E, dtype=np.int64)
        else:
            g[:T] = nbr[:, 0]
            g[T:] = _ext_idx(lv, E)
        lv["g_idx"] = g
        ok = np.ones(T, dtype=bool)
        for k, s in enumerate([0, 1, R, R + 1, R * R, R * R + 1, R * R + R, R * R + R + 1]):
            ok &= nbr[:, k] == g[s:s + T]
        # for non-dense levels, the streamed g values must also lie within the
        # level (the bf16 gather window is this level only)
        if not lv["dense"]:
            inlvl = (g >= off) & (g < off + T)
            for s in [0, 1, R, R + 1, R * R, R * R + 1, R * R + R, R * R + R + 1]:
                ok &= inlvl[s:s + T]
        lv["ok"] = ok
        lv["E"] = E

    # per-level segment meta (uniform across cores)
    segs = []
    for li, lv in enumerate(levels):
        PL = -(-lv["chunk"] // P)
        mode = "dense" if lv["dense"] else ("two" if lv["R"] >= TWO_STREAM_MIN_R else "one")
        segs.append(dict(li=li, R=lv["R"], PL=PL, mode=mode,
                         off=lv["off"], T=lv["T"], chunk=lv["chunk"]))

    # fixups per (core, level) and hard fixups (neighbors outside own level)
    fix = [[[] for _ in levels] for _ in range(NC)]
    hard = [[] for _ in range(NC)]
    for li, lv in enumerate(levels):
        off, T = lv["off"], lv["T"]
        bad = np.nonzero(~lv["ok"])[0]
        if len(bad) == 0:
            continue
        nb = neighbor_idx[off + bad]  # [nbad, 8]
        # fixup rows must come from this level's bf16 window (dense: own level
        # f32 path also gathers from the level's bf16 window)
        ok_here = ((nb >= off) & (nb < off + T)).all(axis=1)
        for j, oh in zip(bad, ok_here):
            c = min(int(j // lv["chunk"]), NC - 1)
            if oh:
                fix[c][li].append(int(j))
            else:
                hard[c].append(off + int(j))

    # uniform fixup quotas per level
    fq = []
    for li in range(len(levels)):
        mx = max(len(fix[c][li]) for c in range(NC))
        fq.append(-(-max(mx, 1) // P) * P)
    nhard_max = max(len(h) for h in hard)
    HQ = -(-max(nhard_max, 0) // P) * P  # hard quota (0 if none)

    # gather stream layout per level (positions in "w" space), uniform:
    #   one:   [0, 128*PL + E)            main window
    #   two:   [0, L0) main, [L0, L0+L1) +R^2 window
    #   dense: no main window
    # then fixup region: 8 * fq[li] positions, laid out per-partition:
    #   partition p, slot j, neighbor k -> fixoff + p*(q*8) + j*8 + k
    goff = 0
    for sm in segs:
        R, PL, li = sm["R"], sm["PL"], sm["li"]
        sm["g0"] = goff
        if sm["mode"] == "one":
            sm["len0"] = P * PL + R * R + R + 2
            sm["len1"] = 0
        elif sm["mode"] == "two":
            sm["len0"] = P * PL + R + 2
            sm["len1"] = P * PL + R + 2
        else:
            sm["len0"] = 0
            sm["len1"] = 0
        sm["fixoff"] = sm["len0"] + sm["len1"]
        sm["q"] = fq[li] // P
        slen = sm["fixoff"] + 8 * fq[li]
        slen = -(-slen // (P * CT)) * (P * CT)  # pad to gather-tile multiple
        sm["slen"] = slen
        sm["ntiles"] = slen // (P * CT)
        goff += slen
    GTOT = goff  # gbuf rows per core

    # out layout
    ooff = 0
    for sm in segs:
        sm["o0"] = ooff
        ooff += P * sm["PL"]
    OUT_ROWS = ooff + P  # + dummy tail for fixup padding + hard pads

    return dict(levels=levels, segs=segs, fix=fix, hard=hard, fq=fq, HQ=HQ,
                GTOT=GTOT, OUT_ROWS=OUT_ROWS, N=N)


def _core_arrays(plan, neighbor_idx, c):
    """Build per-core gather idx (int16 blocks), masks (bf16), scatter idx."""
    segs, levels = plan["segs"], plan["levels"]
    rows = np.zeros(plan["GTOT"], dtype=np.int64)   # global row per position
    valid = np.zeros(plan["GTOT"], dtype=bool)
    scat = []
    for sm in segs:
        lv = levels[sm["li"]]
        off, T, R = lv["off"], lv["T"], sm["R"]
        es = c * sm["chunk"]
        g = lv["g_idx"]
        base = sm["g0"]

        def put(dst, start, length):
            s = max(0, min(start, len(g)))
            e = max(0, min(start + length, len(g)))
            if e > s:
                rows[dst + (s - start): dst + (e - start)] = g[s:e]
                valid[dst + (s - start): dst + (e - start)] = True

        if sm["mode"] == "one":
            put(base, es, sm["len0"])
        elif sm["mode"] == "two":
            put(base, es, sm["len0"])
            put(base + sm["len0"], es + R * R, sm["len1"])
        # fixups
        fxs = plan["fix"][c][sm["li"]]
        q = sm["q"]
        soff = np.full((P, q), plan["OUT_ROWS"] - 1, dtype=np.int32)
        for f, j in enumerate(fxs):
            p, jj = f // q, f % q
            w = base + sm["fixoff"] + p * (q * 8) + jj * 8
            rows[w:w + 8] = neighbor_idx[off + j]
            valid[w:w + 8] = True
            soff[p, jj] = sm["o0"] + j
        scat.append(soff)
        # positions of this level must gather within [off, off+T): map invalid
        # to off (block 0)
        lo, hi = base, base + sm["slen"]
        r = rows[lo:hi]
        v = valid[lo:hi]
        r[~v] = off
        np.clip(r, off, off + T - 1, out=r)
        rows[lo:hi] = r

    # per-level: block idx (within level) + sub code, then interleave-feed order
    gidx = np.zeros((plan["GTOT"] // (P * CT), P, CT * 8), dtype=np.int16)
    msk = np.zeros((plan["GTOT"] // (P * CT), P, CT * 16), dtype=_bf16)
    tglob = 0
    for sm in segs:
        lv = levels[sm["li"]]
        lo = sm["g0"]
        GL = sm["slen"] // P
        r = rows[lo:lo + sm["slen"]] - lv["off"]
        blk = (r // BPB).astype(np.int16)
        sub = (r % BPB).astype(np.int16)
        blk_m = blk.reshape(P, GL)
        sub_m = sub.reshape(P, GL)
        for t in range(sm["ntiles"]):
            bt = blk_m[:, t * CT:(t + 1) * CT]          # [P, CT] output-layout
            feed = bt.T.reshape(-1)                     # feed order: i -> (p=i%128, c=i//128)
            w = feed.reshape(CT * 8, 16).T              # wrapped [16, n/16]
            gidx[tglob, :, :] = np.tile(w, (8, 1))
            st = sub_m[:, t * CT:(t + 1) * CT]          # [P, CT]
            m = np.zeros((P, CT, 16), dtype=_bf16)
            np.put_along_axis(m, st[:, :, None].astype(np.int64), _bf16(1.0), axis=2)
            msk[tglob] = m.reshape(P, CT * 16)
            tglob += 1

    scat_all = np.concatenate([s.reshape(P, -1) for s in scat], axis=1)  # [P, sum q]
    # hard fixups
    hq = plan["HQ"]
    hrows = np.zeros((max(hq, 1), 8), dtype=np.int32)
    hout = np.full(max(hq, 1), plan["OUT_ROWS"] - 1, dtype=np.int32)
    segs_by_entry = plan["hard"][c]
    for f, ge in enumerate(segs_by_entry):
        hrows[f] = neighbor_idx[ge]
        # locate out row: level + local j
        for sm in segs:
            lv = levels[sm["li"]]
            if lv["off"] <= ge < lv["off"] + lv["T"]:
                j = ge - lv["off"] - c * sm["chunk"]
                hout[f] = sm["o0"] + j
                break
    return gidx, msk, scat_all.astype(np.int32), hrows, hout


def _build_nc(plan, NT, NQS, dense_rows):
    """Build the uniform Bass program."""
    segs = plan["segs"]
    nc = bacc.Bacc("TRN2", target_bir_lowering=False, debug=False, num_devices=NC)
    f32, bf16, i16, i32 = (mybir.dt.float32, mybir.dt.bfloat16,
                           mybir.dt.int16, mybir.dt.int32)
    N = plan["N"]
    xb = nc.dram_tensor("xb", [N, 8], bf16, kind="ExternalInput")
    xd = nc.dram_tensor("xd", [dense_rows, 8], f32, kind="ExternalInput")
    gidx = nc.dram_tensor("gidx", [NT, P, CT * 8], i16, kind="ExternalInput")
    mskd = nc.dram_tensor("mskd", [NT, P, CT * 16], bf16, kind="ExternalInput")
    wti = nc.dram_tensor("wt", [P, 64], f32, kind="ExternalInput")
    bti = nc.dram_tensor("bt", [P, 8], f32, kind="ExternalInput")
    scat = nc.dram_tensor("scat", [P, NQS], i32, kind="ExternalInput")
    HQ = plan["HQ"]
    if HQ:
        hrowst = nc.dram_tensor("hrows", [HQ, 8], i32, kind="ExternalInput")
        houtt = nc.dram_tensor("hout", [HQ], i32, kind="ExternalInput")
    out = nc.dram_tensor("out", [plan["OUT_ROWS"], 8], f32, kind="ExternalOutput")
    nfx_cols = NQS * 8 + (plan["HQ"] // P) * 8
    fxo = nc.dram_tensor("fxo", [P, max(nfx_cols, 8)], f32, kind="ExternalOutput")
    gbuf = nc.dram_tensor("gbuf", [plan["GTOT"] * 8], f32)

    xbf = xb.ap().rearrange("a b -> (a b)")
    gb = gbuf.ap()
    xdf = xd.ap().rearrange("a b -> (a b)")
    outf = out.ap().rearrange("a b -> (a b)")

    with TileContext(nc) as tc:
        with (
            tc.tile_pool(name="const", bufs=1) as constp,
            tc.tile_pool(name="pa", bufs=2) as pa,
            tc.tile_pool(name="pasc", bufs=1) as pasc,
            tc.tile_pool(name="pag", bufs=3) as pag,
            tc.tile_pool(name="pb", bufs=2) as pb,
            tc.tile_pool(name="pbt", bufs=2) as pbt,
        ):
            wt = constp.tile([P, 64], f32)
            bt = constp.tile([P, 8], f32)
            nc.sync.dma_start(out=wt[:], in_=wti[:])
            nc.sync.dma_start(out=bt[:], in_=bti[:])

            # Emission order: dense streaming first (no gather deps), then
            # per hashed level gather -> extract -> stream -> fixups, so the
            # scheduler can overlap level l's streaming with level l+1's
            # gathers instead of serializing whole phases.

            def emit_pa(sm):
                lv = plan["levels"][sm["li"]]
                nblk = -(-lv["T"] // BPB)
                win = bass.AP(xb, lv["off"] * 8, [[128, nblk], [1, 128]])
                GL = sm["slen"] // P
                for t in range(sm["ntiles"]):
                    tg = sm["tile_base"] + t
                    idx_sb = pa.tile([P, CT * 8], i16, tag="idx")
                    nc.scalar.dma_start(out=idx_sb[:], in_=gidx[tg])
                    mk = pa.tile([P, CT * 16], bf16, tag="msk")
                    nc.scalar.dma_start(out=mk[:], in_=mskd[tg])
                    gat = pag.tile([P, CT * 128], bf16, tag="gat")
                    nc.gpsimd.dma_gather(
                        out_ap=gat[:].rearrange("p (c e) -> p c e", e=128),
                        in_ap=win,
                        idxs_ap=idx_sb[:],
                        num_idxs=P * CT,
                        num_idxs_reg=P * CT,
                        elem_size=128,
                        single_packet=False,
                    )
                    # one-hot mask multiply with a fully contiguous write,
                    # then a contiguous tree-reduction over the 16 sub-rows
                    # (masks are one-hot, so bf16 partials are exact).
                    tmp = pasc.tile([P, CT * 128], bf16, tag="tmp")
                    in0 = gat[:].rearrange("p (cs e) -> p cs e", e=8)
                    in1 = bass.AP(mk[:].tensor, mk[:].offset,
                                  [mk[:].ap[0], [1, CT * 16], [0, 8]])
                    outv = tmp[:].rearrange("p (cs e) -> p cs e", e=8)
                    nc.vector.tensor_tensor(out=outv, in0=in0, in1=in1,
                                            op=mybir.AluOpType.mult)
                    a1 = pasc.tile([P, CT * 64], bf16, tag="a1")
                    nc.vector.tensor_tensor(
                        out=a1[:],
                        in0=bass.AP(tmp[:].tensor, tmp[:].offset,
                                    [tmp[:].ap[0], [128, CT], [1, 64]]),
                        in1=bass.AP(tmp[:].tensor, tmp[:].offset + 64,
                                    [tmp[:].ap[0], [128, CT], [1, 64]]),
                        op=mybir.AluOpType.add)
                    a2 = pasc.tile([P, CT * 32], bf16, tag="a2")
                    nc.vector.tensor_tensor(
                        out=a2[:],
                        in0=bass.AP(a1[:].tensor, a1[:].offset,
                                    [a1[:].ap[0], [64, CT], [1, 32]]),
                        in1=bass.AP(a1[:].tensor, a1[:].offset + 32,
                                    [a1[:].ap[0], [64, CT], [1, 32]]),
                        op=mybir.AluOpType.add)
                    a3 = pasc.tile([P, CT * 16], bf16, tag="a1")
                    nc.vector.tensor_tensor(
                        out=a3[:],
                        in0=bass.AP(a2[:].tensor, a2[:].offset,
                                    [a2[:].ap[0], [32, CT], [1, 16]]),
                        in1=bass.AP(a2[:].tensor, a2[:].offset + 16,
                                    [a2[:].ap[0], [32, CT], [1, 16]]),
                        op=mybir.AluOpType.add)
                    rows_t = pa.tile([P, CT * 8], f32, tag="rows")
                    nc.vector.tensor_tensor(
                        out=rows_t[:],
                        in0=bass.AP(a3[:].tensor, a3[:].offset,
                                    [a3[:].ap[0], [16, CT], [1, 8]]),
                        in1=bass.AP(a3[:].tensor, a3[:].offset + 8,
                                    [a3[:].ap[0], [16, CT], [1, 8]]),
                        op=mybir.AluOpType.add)
                    dst = bass.AP(gbuf, (sm["g0"] + t * CT) * 8,
                                  [[GL * 8, P], [1, CT * 8]])
                    nc.sync.dma_start(out=dst, in_=rows_t[:])

            def affine_store(srcacc, n, dest_dma):
                """srcacc: f32 [P, n*8] neighbor-sum tile -> (y@W'+b), then
                dest_dma(result_tile, n)."""
                ab = pb.tile([P, n * 8], bf16, tag="ybf")
                nc.vector.tensor_copy(out=ab[:, :n * 8], in_=srcacc)
                ot = pb.tile([P, n * 8], f32, tag="ot")
                for h in range(2):
                    prod = pb.tile([P, n * 32], bf16, tag="prod")
                    i0 = ab[:, :n * 8].rearrange("p (c e) -> p c e", e=8)
                    i0 = bass.AP(i0.tensor, i0.offset,
                                 [i0.ap[0], i0.ap[1], [0, 4], i0.ap[2]])
                    wv = wt[:, h * 32:(h + 1) * 32].rearrange("p (o e) -> p o e", e=8)
                    i1 = bass.AP(wv.tensor, wv.offset,
                                 [wv.ap[0], [0, n], wv.ap[1], wv.ap[2]])
                    pv = prod[:, :n * 32].rearrange("p (c o e) -> p c o e", o=4, e=8)
                    nc.vector.tensor_tensor(out=pv, in0=i0, in1=i1,
                                            op=mybir.AluOpType.mult)
                    nc.vector.reduce_sum(
                        out=bass.AP(ot[:].tensor, ot[:].offset + h * 4,
                                    [ot[:].ap[0], [8, n], [1, 4]]),
                        in_=prod[:, :n * 32].rearrange("p (co e) -> p co e", e=8),
                        axis=mybir.AxisListType.X)
                bv = bass.AP(bt[:].tensor, bt[:].offset, [bt[:].ap[0], [0, n], [1, 8]])
                ov = ot[:, :n * 8].rearrange("p (c e) -> p c e", e=8)
                nc.vector.tensor_tensor(out=ov, in0=ov, in1=bv,
                                        op=mybir.AluOpType.add)
                dest_dma(ot)

            def emit_pb(sm):
                R, PL = sm["R"], sm["PL"]
                lv = plan["levels"][sm["li"]]
                if sm["mode"] == "dense":
                    src, sbase = xd, lv["off"] * 8
                else:
                    src, sbase = gbuf, sm["g0"] * 8
                nchunk = -(-PL // CB)
                for k in range(nchunk):
                    w = min(CB, PL - k * CB)
                    WN = w + R + 2
                    t0 = pbt.tile([P, WN * 8], f32, tag="t")
                    t1 = pbt.tile([P, WN * 8], f32, tag="t")
                    if sm["mode"] == "two":
                        a0 = bass.AP(src, sbase + k * CB * 8,
                                     [[PL * 8, P], [1, WN * 8]])
                        a1 = bass.AP(src, (sm["g0"] + sm["len0"] + k * CB) * 8,
                                     [[PL * 8, P], [1, WN * 8]])
                    else:
                        a0 = bass.AP(src, sbase + k * CB * 8,
                                     [[PL * 8, P], [1, WN * 8]])
                        a1 = bass.AP(src, sbase + (k * CB + R * R) * 8,
                                     [[PL * 8, P], [1, WN * 8]])
                    nc.sync.dma_start(out=t0[:], in_=a0)
                    nc.sync.dma_start(out=t1[:], in_=a1)
                    nc.vector.tensor_tensor(out=t0[:], in0=t0[:], in1=t1[:],
                                            op=mybir.AluOpType.add)
                    UN = w + R
                    nc.vector.tensor_tensor(out=t1[:, :UN * 8],
                                            in0=t0[:, :UN * 8],
                                            in1=t0[:, 8:(UN + 1) * 8],
                                            op=mybir.AluOpType.add)
                    y = pb.tile([P, CB * 8], f32, tag="y")
                    nc.vector.tensor_tensor(out=y[:, :w * 8], in0=t1[:, :w * 8],
                                            in1=t1[:, R * 8:(w + R) * 8],
                                            op=mybir.AluOpType.add)

                    def dest(ot, sm=sm, k=k, w=w, PL=PL):
                        od = bass.AP(out, (sm["o0"] + k * CB) * 8,
                                     [[PL * 8, P], [1, w * 8]])
                        nc.sync.dma_start(out=od, in_=ot[:, :w * 8])
                    affine_store(y[:, :w * 8], w, dest)

            def emit_pc(sm, qpos):
                q = sm["q"]
                if q == 0:
                    return
                fr = pb.tile([P, q * 64], f32, tag="fr")
                a = bass.AP(gbuf, (sm["g0"] + sm["fixoff"]) * 8,
                            [[q * 64, P], [1, q * 64]])
                nc.sync.dma_start(out=fr[:], in_=a)
                acc = pb.tile([P, q * 8], f32, tag="facc")
                v = fr[:].rearrange("p (j k e) -> p j k e", k=8, e=8)
                nc.vector.tensor_copy(out=acc[:].rearrange("p (j e) -> p j e", e=8),
                                      in_=v[:, :, 0, :])
                for kk in range(1, 8):
                    nc.vector.tensor_tensor(
                        out=acc[:].rearrange("p (j e) -> p j e", e=8),
                        in0=acc[:].rearrange("p (j e) -> p j e", e=8),
                        in1=v[:, :, kk, :], op=mybir.AluOpType.add)

                def dest(ot, qpos=qpos, q=q):
                    nc.sync.dma_start(out=fxo[:, qpos * 8:(qpos + q) * 8],
                                      in_=ot[:, :q * 8])
                affine_store(acc[:], q, dest)

            # dense streaming first (independent of all gathers)
            for sm in segs:
                if sm["mode"] == "dense":
                    emit_pb(sm)
            # per-level pipeline
            qpos_map = {}
            qpos = 0
            for sm in segs:
                qpos_map[sm["li"]] = qpos
                qpos += sm["q"]
            for sm in segs:
                if sm["ntiles"]:
                    emit_pa(sm)
                if sm["mode"] != "dense":
                    emit_pb(sm)
                emit_pc(sm, qpos_map[sm["li"]])

            # ---- hard fixups (rare; neighbors cross levels) ----
            if HQ:
                nh = HQ // P
                hi = pb.tile([P, nh * 8], i32, tag="hi")
                nc.sync.dma_start(out=hi[:], in_=hrowst.ap().rearrange(
                    "(a p) b -> p (a b)", p=P))
                hacc = pb.tile([P, nh * 8], f32, tag="hacc")
                hrow = pb.tile([P, 8], bf16, tag="hrow")
                for j in range(nh):
                    for kk in range(8):
                        nc.gpsimd.indirect_dma_start(
                            out=hrow[:], out_offset=None, in_=xb.ap(),
                            in_offset=bass.IndirectOffsetOnAxis(
                                ap=hi[:, j * 8 + kk:j * 8 + kk + 1], axis=0))
                        if kk == 0:
                            nc.vector.tensor_copy(out=hacc[:, j * 8:(j + 1) * 8], in_=hrow[:])
                        else:
                            nc.vector.tensor_tensor(
                                out=hacc[:, j * 8:(j + 1) * 8],
                                in0=hacc[:, j * 8:(j + 1) * 8], in1=hrow[:],
                                op=mybir.AluOpType.add)

                def hdest(ot, nh=nh):
                    nc.sync.dma_start(out=fxo[:, NQS * 8:(NQS + nh) * 8],
                                      in_=ot[:, :nh * 8])
                affine_store(hacc[:], nh, hdest)
    nc.compile()
    return nc


def kernel(x, W, b, neighbor_idx):
    x = np.asarray(x)
    W = np.asarray(W, dtype=np.float32)
    b = np.asarray(b, dtype=np.float32)
    neighbor_idx = np.asarray(neighbor_idx, dtype=np.int64)
    in_dtype = x.dtype
    x2 = x.reshape(x.shape[0], -1).astype(np.float32)
    N = x2.shape[0]

    plan = _plan(neighbor_idx)
    segs = plan["segs"]
    tb = 0
    for sm in segs:
        sm["tile_base"] = tb
        tb += sm["ntiles"]
        if sm["mode"] == "dense":
            sm["es8"] = 0  # per-core entry start handled via xd slice offset
    NT = tb

    # bf16 table (round-to-nearest-ish)
    xb_bits = ((x2.view(np.uint32) + 0x8000) >> 16).astype(np.uint16)
    xbf = xb_bits.view(_bf16).reshape(N, 8)
    dense_end = max(sm["off"] + plan["levels"][sm["li"]]["T"] for sm in segs if sm["mode"] == "dense")
    dense_rows = min(N, dense_end + 300000)

    per_core = []
    NQS = sum(sm["q"] for sm in segs)
    for c in range(NC):
        gidx, msk, scat, hrows, hout = _core_arrays(plan, neighbor_idx, c)
        # dense phase-B reads xd at per-core offsets: we shift the dense data
        # per core instead (xd differs per core)
        xd = np.zeros((dense_rows, 8), dtype=np.float32)
        for sm in segs:
            if sm["mode"] != "dense":
                continue
            lv = plan["levels"][sm["li"]]
            es = c * sm["chunk"]
            lo = lv["off"] + es
            hi = min(N, lo + P * sm["PL"] + sm["R"] ** 2 + sm["R"] + 2)
            xd[lv["off"]:lv["off"] + (hi - lo)] = x2[lo:hi]
        wt = np.tile((W / 8.0).reshape(1, 64), (P, 1)).astype(np.float32)
        bt = np.tile(b.reshape(1, 8), (P, 1)).astype(np.float32)
        m = dict(xb=np.ascontiguousarray(xbf), xd=xd, gidx=gidx, mskd=msk,
                 wt=wt, bt=bt, scat=scat)
        if plan["HQ"]:
            m["hrows"] = hrows[:plan["HQ"]]
            m["hout"] = hout[:plan["HQ"]]
        per_core.append(m)

    nc = _build_nc(plan, NT, NQS, dense_rows)
    kernel.last_nc = nc
    kernel.last_per_core = per_core
    import time as _time
    _t0 = _time.time()
    res = run_bass_kernel_spmd(nc, per_core, list(range(NC)))
    kernel.last_spmd_wall_s = _time.time() - _t0

    full = np.empty((N, 8), dtype=np.float32)
    for c in range(NC):
        co = res.results[c]["out"]
        for sm in segs:
            lv = plan["levels"][sm["li"]]
            es = c * sm["chunk"]
            ecount = min(sm["chunk"], lv["T"] - es)
            if ecount <= 0:
                continue
            full[lv["off"] + es: lv["off"] + es + ecount] = co[sm["o0"]: sm["o0"] + ecount]
    # overlay device-computed fixup rows (host does placement only)
    for c in range(NC):
        fx = res.results[c]["fxo"]
        qpos = 0
        for sm in segs:
            lv = plan["levels"][sm["li"]]
            q = sm["q"]
            fxs = plan["fix"][c][sm["li"]]
            for f, j in enumerate(fxs):
                p, jj = f // q, f % q
                full[lv["off"] + j] = fx[p, (qpos + jj) * 8:(qpos + jj + 1) * 8]
            qpos += q
        if plan["HQ"]:
            nh = plan["HQ"] // P
            for f, ge in enumerate(plan["hard"][c]):
                p, jj = f % P, f // P
                full[ge] = fx[p, (NQS + jj) * 8:(NQS + jj + 1) * 8]
    return full.reshape(x.shape).astype(in_dtype)


# revision 13
# speedup vs baseline: 10436.7112x; 1.7801x over previous
"""Trainium2 Bass kernel for the multi-level hash-grid context layer.

Algorithm (corner-stream): for the instant-NGP neighbor structure,
neighbor k of entry i equals the corner hash of entry i+s_k for 8 fixed
shifts s_k = {0,1,R,R+1,R^2,R^2+1,R^2+R,R^2+R+1}. So instead of 8 random
gathers per entry we gather ONE corner stream g[j] = x[nbr0[j]] (+ a small
structure-derived extension) and compute the 8-neighbor sum with shifted
streaming adds. Host-side verification compares every entry's actual
neighbor indices against the streamed relation; mismatches (level
boundaries / clamps / arbitrary inputs) are fixed up with direct gathers.

Gather engine: InstDMAGatherAnt (bulk 256B-block gather, int16 indices)
from a bf16 copy of the table; sub-row (16 bf16 rows per 256B block)
extracted on-chip with one-hot masks + reduce. Dense levels (R^3 <= 2^19)
skip the gather entirely (corner stream is the identity there).

All 8 cores run one uniform program (SPMD); each core owns 1/8 of every
level. Work is data-parameterized per core.
"""
import numpy as np
import ml_dtypes

import concourse.bass as bass
import concourse.bacc as bacc
import concourse.mybir as mybir
from concourse.tile import TileContext, add_dep_helper
from concourse.bass_utils import run_bass_kernel_spmd

RES = [16, 20, 25, 32, 40, 51, 64, 81, 102, 128, 161, 203, 256, 323, 406, 512]
CAP = 1 << 19
PRIMES = np.array([1, 2654435761, 805459861], dtype=np.uint32)
NC = 8           # cores
P = 128          # partitions
CT = 64          # gather positions per partition per tile
CB = 192         # phase-B chunk columns (entries per partition per chunk)
BPB = 16         # bf16 rows per 256B gather block
TWO_STREAM_MIN_R = 300

_bf16 = ml_dtypes.bfloat16


def _levels():
    sizes = [min(r ** 3, CAP) for r in RES]
    offs = np.concatenate([[0], np.cumsum(sizes)]).astype(np.int64)
    out = []
    for i, r in enumerate(RES):
        out.append(dict(R=r, T=sizes[i], off=int(offs[i]), dense=r ** 3 <= CAP,
                        chunk=-(-sizes[i] // NC)))
    return out, int(offs[-1])


def _ext_idx(lv, count):
    R = lv["R"]
    j = np.arange(lv["T"], lv["T"] + count, dtype=np.int64)
    cx, cy, cz = (j // (R * R)) % R, (j // R) % R, j % R
    h = (cx.astype(np.uint32) * PRIMES[0]) ^ (cy.astype(np.uint32) * PRIMES[1]) ^ \
        (cz.astype(np.uint32) * PRIMES[2])
    return (lv["off"] + (h % np.uint32(CAP)).astype(np.int64)).astype(np.int64)


def _plan(neighbor_idx):
    levels, N = _levels()
    for lv in levels:
        off, T, R = lv["off"], lv["T"], lv["R"]
        nbr = neighbor_idx[off:off + T]
        E = R * R + R + 2
        g = np.empty(T + E, dtype=np.int64)
        if lv["dense"]:
            g[:] = off + np.arange(T + E, dtype=np.int64)
        else:
            g[:T] = nbr[:, 0]
            g[T:] = _ext_idx(lv, E)
        lv["g_idx"] = g
        ok = np.ones(T, dtype=bool)
        for k, s in enumerate([0, 1, R, R + 1, R * R, R * R + 1, R * R + R, R * R + R + 1]):
            ok &= nbr[:, k] == g[s:s + T]
        # for non-dense levels, the streamed g values must also lie within the
        # level (the bf16 gather window is this level only)
        if not lv["dense"]:
            inlvl = (g >= off) & (g < off + T)
            for s in [0, 1, R, R + 1, R * R, R * R + 1, R * R + R, R * R + R + 1]:
                ok &= inlvl[s:s + T]
        lv["ok"] = ok
        lv["E"] = E

    # per-level segment meta (uniform across cores)
    segs = []
    for li, lv in enumerate(levels):
        PL = -(-lv["chunk"] // P)
        mode = "dense" if lv["dense"] else ("two" if lv["R"] >= TWO_STREAM_MIN_R else "one")
        segs.append(dict(li=li, R=lv["R"], PL=PL, mode=mode,
                         off=lv["off"], T=lv["T"], chunk=lv["chunk"]))

    # fixups per (core, level) and hard fixups (neighbors outside own level)
    fix = [[[] for _ in levels] for _ in range(NC)]
    hard = [[] for _ in range(NC)]
    for li, lv in enumerate(levels):
        off, T = lv["off"], lv["T"]
        bad = np.nonzero(~lv["ok"])[0]
        if len(bad) == 0:
            continue
        nb = neighbor_idx[off + bad]  # [nbad, 8]
        # fixup rows must come from this level's bf16 window (dense: own level
        # f32 path also gathers from the level's bf16 window)
        ok_here = ((nb >= off) & (nb < off + T)).all(axis=1)
        for j, oh in zip(bad, ok_here):
            c = min(int(j // lv["chunk"]), NC - 1)
            if oh:
                fix[c][li].append(int(j))
            else:
                hard[c].append(off + int(j))

    # uniform fixup quotas per level
    fq = []
    for li in range(len(levels)):
        mx = max(len(fix[c][li]) for c in range(NC))
        fq.append(-(-max(mx, 1) // P) * P)
    nhard_max = max(len(h) for h in hard)
    HQ = -(-max(nhard_max, 0) // P) * P  # hard quota (0 if none)

    # gather stream layout per level (positions in "w" space), uniform:
    #   one:   [0, 128*PL + E)            main window
    #   two:   [0, L0) main, [L0, L0+L1) +R^2 window
    #   dense: no main window
    # then fixup region: 8 * fq[li] positions, laid out per-partition:
    #   partition p, slot j, neighbor k -> fixoff + p*(q*8) + j*8 + k
    goff = 0
    for sm in segs:
        R, PL, li = sm["R"], sm["PL"], sm["li"]
        sm["g0"] = goff
        if sm["mode"] == "one":
            sm["len0"] = P * PL + R * R + R + 2
            sm["len1"] = 0
        elif sm["mode"] == "two":
            sm["len0"] = P * PL + R + 2
            sm["len1"] = P * PL + R + 2
        else:
            sm["len0"] = 0
            sm["len1"] = 0
        sm["fixoff"] = sm["len0"] + sm["len1"]
        sm["q"] = fq[li] // P
        slen = sm["fixoff"] + 8 * fq[li]
        slen = -(-slen // (P * CT)) * (P * CT)  # pad to gather-tile multiple
        sm["slen"] = slen
        sm["ntiles"] = slen // (P * CT)
        goff += slen
    GTOT = goff  # gbuf rows per core

    # out layout
    ooff = 0
    for sm in segs:
        sm["o0"] = ooff
        ooff += P * sm["PL"]
    OUT_ROWS = ooff + P  # + dummy tail for fixup padding + hard pads

    return dict(levels=levels, segs=segs, fix=fix, hard=hard, fq=fq, HQ=HQ,
                GTOT=GTOT, OUT_ROWS=OUT_ROWS, N=N)


def _core_arrays(plan, neighbor_idx, c):
    """Build per-core gather idx (int16 blocks), masks (bf16), scatter idx."""
    segs, levels = plan["segs"], plan["levels"]
    rows = np.zeros(plan["GTOT"], dtype=np.int64)   # global row per position
    valid = np.zeros(plan["GTOT"], dtype=bool)
    scat = []
    for sm in segs:
        lv = levels[sm["li"]]
        off, T, R = lv["off"], lv["T"], sm["R"]
        es = c * sm["chunk"]
        g = lv["g_idx"]
        base = sm["g0"]

        def put(dst, start, length):
            s = max(0, min(start, len(g)))
            e = max(0, min(start + length, len(g)))
            if e > s:
                rows[dst + (s - start): dst + (e - start)] = g[s:e]
                valid[dst + (s - start): dst + (e - start)] = True

        if sm["mode"] == "one":
            put(base, es, sm["len0"])
        elif sm["mode"] == "two":
            put(base, es, sm["len0"])
            put(base + sm["len0"], es + R * R, sm["len1"])
        # fixups
        fxs = plan["fix"][c][sm["li"]]
        q = sm["q"]
        soff = np.full((P, q), plan["OUT_ROWS"] - 1, dtype=np.int32)
        for f, j in enumerate(fxs):
            p, jj = f // q, f % q
            w = base + sm["fixoff"] + p * (q * 8) + jj * 8
            rows[w:w + 8] = neighbor_idx[off + j]
            valid[w:w + 8] = True
            soff[p, jj] = sm["o0"] + j
        scat.append(soff)
        # positions of this level must gather within [off, off+T): map invalid
        # to off (block 0)
        lo, hi = base, base + sm["slen"]
        r = rows[lo:hi]
        v = valid[lo:hi]
        r[~v] = off
        np.clip(r, off, off + T - 1, out=r)
        rows[lo:hi] = r

    # per-level: block idx (within level) + sub code, then interleave-feed order
    gidx = np.zeros((plan["GTOT"] // (P * CT), P, CT * 8), dtype=np.int16)
    msk = np.zeros((plan["GTOT"] // (P * CT), P, CT * 16), dtype=_bf16)
    tglob = 0
    for sm in segs:
        lv = levels[sm["li"]]
        lo = sm["g0"]
        GL = sm["slen"] // P
        r = rows[lo:lo + sm["slen"]] - lv["off"]
        blk = (r // BPB).astype(np.int16)
        sub = (r % BPB).astype(np.int16)
        blk_m = blk.reshape(P, GL)
        sub_m = sub.reshape(P, GL)
        for t in range(sm["ntiles"]):
            bt = blk_m[:, t * CT:(t + 1) * CT]          # [P, CT] output-layout
            feed = bt.T.reshape(-1)                     # feed order: i -> (p=i%128, c=i//128)
            w = feed.reshape(CT * 8, 16).T              # wrapped [16, n/16]
            gidx[tglob, :, :] = np.tile(w, (8, 1))
            st = sub_m[:, t * CT:(t + 1) * CT]          # [P, CT]
            m = np.zeros((P, CT, 16), dtype=_bf16)
            np.put_along_axis(m, st[:, :, None].astype(np.int64), _bf16(1.0), axis=2)
            msk[tglob] = m.reshape(P, CT * 16)
            tglob += 1

    scat_all = np.concatenate([s.reshape(P, -1) for s in scat], axis=1)  # [P, sum q]
    # hard fixups
    hq = plan["HQ"]
    hrows = np.zeros((max(hq, 1), 8), dtype=np.int32)
    hout = np.full(max(hq, 1), plan["OUT_ROWS"] - 1, dtype=np.int32)
    segs_by_entry = plan["hard"][c]
    for f, ge in enumerate(segs_by_entry):
        hrows[f] = neighbor_idx[ge]
        # locate out row: level + local j
        for sm in segs:
            lv = levels[sm["li"]]
            if lv["off"] <= ge < lv["off"] + lv["T"]:
                j = ge - lv["off"] - c * sm["chunk"]
                hout[f] = sm["o0"] + j
                break
    return gidx, msk, scat_all.astype(np.int32), hrows, hout


def _build_nc(plan, NT, NQS, dense_rows):
    """Build the uniform Bass program."""
    segs = plan["segs"]
    nc = bacc.Bacc("TRN2", target_bir_lowering=False, debug=False, num_devices=NC,
                   num_swdge_queues=4)
    f32, bf16, i16, i32 = (mybir.dt.float32, mybir.dt.bfloat16,
                           mybir.dt.int16, mybir.dt.int32)
    N = plan["N"]
    xb = nc.dram_tensor("xb", [N, 8], bf16, kind="ExternalInput")
    xd = nc.dram_tensor("xd", [dense_rows, 8], f32, kind="ExternalInput")
    gidx = nc.dram_tensor("gidx", [NT, P, CT * 8], i16, kind="ExternalInput")
    mskd = nc.dram_tensor("mskd", [NT, P, CT * 16], bf16, kind="ExternalInput")
    wti = nc.dram_tensor("wt", [P, 64], f32, kind="ExternalInput")
    bti = nc.dram_tensor("bt", [P, 8], f32, kind="ExternalInput")
    scat = nc.dram_tensor("scat", [P, NQS], i32, kind="ExternalInput")
    HQ = plan["HQ"]
    if HQ:
        hrowst = nc.dram_tensor("hrows", [HQ, 8], i32, kind="ExternalInput")
        houtt = nc.dram_tensor("hout", [HQ], i32, kind="ExternalInput")
    out = nc.dram_tensor("out", [plan["OUT_ROWS"], 8], f32, kind="ExternalOutput")
    nfx_cols = NQS * 8 + (plan["HQ"] // P) * 8
    fxo = nc.dram_tensor("fxo", [P, max(nfx_cols, 8)], f32, kind="ExternalOutput")
    gbuf = nc.dram_tensor("gbuf", [plan["GTOT"] * 8], f32)

    xbf = xb.ap().rearrange("a b -> (a b)")
    gb = gbuf.ap()
    xdf = xd.ap().rearrange("a b -> (a b)")
    outf = out.ap().rearrange("a b -> (a b)")

    with TileContext(nc) as tc:
        with (
            tc.tile_pool(name="const", bufs=1) as constp,
            tc.tile_pool(name="pa", bufs=2) as pa,
            tc.tile_pool(name="pasc", bufs=1) as pasc,
            tc.tile_pool(name="pag", bufs=3) as pag,
            tc.tile_pool(name="pb", bufs=2) as pb,
            tc.tile_pool(name="pbt", bufs=2) as pbt,
        ):
            wt = constp.tile([P, 64], f32)
            bt = constp.tile([P, 8], f32)
            nc.sync.dma_start(out=wt[:], in_=wti[:])
            nc.sync.dma_start(out=bt[:], in_=bti[:])

            # Emission order: dense streaming first (no gather deps), then
            # per hashed level gather -> extract -> stream -> fixups, so the
            # scheduler can overlap level l's streaming with level l+1's
            # gathers instead of serializing whole phases.

            def emit_pa(sm):
                lv = plan["levels"][sm["li"]]
                nblk = -(-lv["T"] // BPB)
                win = bass.AP(xb, lv["off"] * 8, [[128, nblk], [1, 128]])
                GL = sm["slen"] // P
                for t in range(sm["ntiles"]):
                    tg = sm["tile_base"] + t
                    idx_sb = pa.tile([P, CT * 8], i16, tag="idx")
                    nc.scalar.dma_start(out=idx_sb[:], in_=gidx[tg])
                    mk = pa.tile([P, CT * 16], bf16, tag="msk")
                    nc.scalar.dma_start(out=mk[:], in_=mskd[tg])
                    gat = pag.tile([P, CT * 128], bf16, tag="gat")
                    nc.gpsimd.dma_gather(
                        out_ap=gat[:].rearrange("p (c e) -> p c e", e=128),
                        in_ap=win,
                        idxs_ap=idx_sb[:],
                        num_idxs=P * CT,
                        num_idxs_reg=P * CT,
                        elem_size=128,
                        single_packet=False,
                        queue_num=t % 4,
                    )
                    # one-hot mask multiply with a fully contiguous write,
                    # then a contiguous tree-reduction over the 16 sub-rows
                    # (masks are one-hot, so bf16 partials are exact).
                    tmp = pasc.tile([P, CT * 128], bf16, tag="tmp")
                    in0 = gat[:].rearrange("p (cs e) -> p cs e", e=8)
                    in1 = bass.AP(mk[:].tensor, mk[:].offset,
                                  [mk[:].ap[0], [1, CT * 16], [0, 8]])
                    outv = tmp[:].rearrange("p (cs e) -> p cs e", e=8)
                    nc.vector.tensor_tensor(out=outv, in0=in0, in1=in1,
                                            op=mybir.AluOpType.mult)
                    a1 = pasc.tile([P, CT * 64], bf16, tag="a1")
                    nc.vector.tensor_tensor(
                        out=a1[:],
                        in0=bass.AP(tmp[:].tensor, tmp[:].offset,
                                    [tmp[:].ap[0], [128, CT], [1, 64]]),
                        in1=bass.AP(tmp[:].tensor, tmp[:].offset + 64,
                                    [tmp[:].ap[0], [128, CT], [1, 64]]),
                        op=mybir.AluOpType.add)
                    a2 = pasc.tile([P, CT * 32], bf16, tag="a2")
                    nc.vector.tensor_tensor(
                        out=a2[:],
                        in0=bass.AP(a1[:].tensor, a1[:].offset,
                                    [a1[:].ap[0], [64, CT], [1, 32]]),
                        in1=bass.AP(a1[:].tensor, a1[:].offset + 32,
                                    [a1[:].ap[0], [64, CT], [1, 32]]),
                        op=mybir.AluOpType.add)
                    a3 = pasc.tile([P, CT * 16], bf16, tag="a1")
                    nc.vector.tensor_tensor(
                        out=a3[:],
                        in0=bass.AP(a2[:].tensor, a2[:].offset,
                                    [a2[:].ap[0], [32, CT], [1, 16]]),
                        in1=bass.AP(a2[:].tensor, a2[:].offset + 16,
                                    [a2[:].ap[0], [32, CT], [1, 16]]),
                        op=mybir.AluOpType.add)
                    rows_t = pa.tile([P, CT * 8], f32, tag="rows")
                    nc.vector.tensor_tensor(
                        out=rows_t[:],
                        in0=bass.AP(a3[:].tensor, a3[:].offset,
                                    [a3[:].ap[0], [16, CT], [1, 8]]),
                        in1=bass.AP(a3[:].tensor, a3[:].offset + 8,
                                    [a3[:].ap[0], [16, CT], [1, 8]]),
                        op=mybir.AluOpType.add)
                    dst = bass.AP(gbuf, (sm["g0"] + t * CT) * 8,
                                  [[GL * 8, P], [1, CT * 8]])
                    nc.sync.dma_start(out=dst, in_=rows_t[:])

            def affine_store(srcacc, n, dest_dma):
                """srcacc: f32 [P, n*8] neighbor-sum tile -> (y@W'+b), then
                dest_dma(result_tile, n)."""
                ab = pb.tile([P, n * 8], bf16, tag="ybf")
                nc.vector.tensor_copy(out=ab[:, :n * 8], in_=srcacc)
                ot = pb.tile([P, n * 8], f32, tag="ot")
                for h in range(2):
                    prod = pb.tile([P, n * 32], bf16, tag="prod")
                    i0 = ab[:, :n * 8].rearrange("p (c e) -> p c e", e=8)
                    i0 = bass.AP(i0.tensor, i0.offset,
                                 [i0.ap[0], i0.ap[1], [0, 4], i0.ap[2]])
                    wv = wt[:, h * 32:(h + 1) * 32].rearrange("p (o e) -> p o e", e=8)
                    i1 = bass.AP(wv.tensor, wv.offset,
                                 [wv.ap[0], [0, n], wv.ap[1], wv.ap[2]])
                    pv = prod[:, :n * 32].rearrange("p (c o e) -> p c o e", o=4, e=8)
                    nc.vector.tensor_tensor(out=pv, in0=i0, in1=i1,
                                            op=mybir.AluOpType.mult)
                    nc.vector.reduce_sum(
                        out=bass.AP(ot[:].tensor, ot[:].offset + h * 4,
                                    [ot[:].ap[0], [8, n], [1, 4]]),
                        in_=prod[:, :n * 32].rearrange("p (co e) -> p co e", e=8),
                        axis=mybir.AxisListType.X)
                bv = bass.AP(bt[:].tensor, bt[:].offset, [bt[:].ap[0], [0, n], [1, 8]])
                ov = ot[:, :n * 8].rearrange("p (c e) -> p c e", e=8)
                nc.vector.tensor_tensor(out=ov, in0=ov, in1=bv,
                                        op=mybir.AluOpType.add)
                dest_dma(ot)

            def emit_pb(sm):
                R, PL = sm["R"], sm["PL"]
                lv = plan["levels"][sm["li"]]
                if sm["mode"] == "dense":
                    src, sbase = xd, lv["off"] * 8
                else:
                    src, sbase = gbuf, sm["g0"] * 8
                nchunk = -(-PL // CB)
                for k in range(nchunk):
                    w = min(CB, PL - k * CB)
                    WN = w + R + 2
                    t0 = pbt.tile([P, WN * 8], f32, tag="t")
                    t1 = pbt.tile([P, WN * 8], f32, tag="t")
                    if sm["mode"] == "two":
                        a0 = bass.AP(src, sbase + k * CB * 8,
                                     [[PL * 8, P], [1, WN * 8]])
                        a1 = bass.AP(src, (sm["g0"] + sm["len0"] + k * CB) * 8,
                                     [[PL * 8, P], [1, WN * 8]])
                    else:
                        a0 = bass.AP(src, sbase + k * CB * 8,
                                     [[PL * 8, P], [1, WN * 8]])
                        a1 = bass.AP(src, sbase + (k * CB + R * R) * 8,
                                     [[PL * 8, P], [1, WN * 8]])
                    nc.sync.dma_start(out=t0[:], in_=a0)
                    nc.sync.dma_start(out=t1[:], in_=a1)
                    nc.vector.tensor_tensor(out=t0[:], in0=t0[:], in1=t1[:],
                                            op=mybir.AluOpType.add)
                    UN = w + R
                    nc.vector.tensor_tensor(out=t1[:, :UN * 8],
                                            in0=t0[:, :UN * 8],
                                            in1=t0[:, 8:(UN + 1) * 8],
                                            op=mybir.AluOpType.add)
                    y = pb.tile([P, CB * 8], f32, tag="y")
                    nc.vector.tensor_tensor(out=y[:, :w * 8], in0=t1[:, :w * 8],
                                            in1=t1[:, R * 8:(w + R) * 8],
                                            op=mybir.AluOpType.add)

                    def dest(ot, sm=sm, k=k, w=w, PL=PL):
                        od = bass.AP(out, (sm["o0"] + k * CB) * 8,
                                     [[PL * 8, P], [1, w * 8]])
                        nc.sync.dma_start(out=od, in_=ot[:, :w * 8])
                    affine_store(y[:, :w * 8], w, dest)

            def emit_pc(sm, qpos):
                q = sm["q"]
                if q == 0:
                    return
                fr = pb.tile([P, q * 64], f32, tag="fr")
                a = bass.AP(gbuf, (sm["g0"] + sm["fixoff"]) * 8,
                            [[q * 64, P], [1, q * 64]])
                nc.sync.dma_start(out=fr[:], in_=a)
                acc = pb.tile([P, q * 8], f32, tag="facc")
                v = fr[:].rearrange("p (j k e) -> p j k e", k=8, e=8)
                nc.vector.tensor_copy(out=acc[:].rearrange("p (j e) -> p j e", e=8),
                                      in_=v[:, :, 0, :])
                for kk in range(1, 8):
                    nc.vector.tensor_tensor(
                        out=acc[:].rearrange("p (j e) -> p j e", e=8),
                        in0=acc[:].rearrange("p (j e) -> p j e", e=8),
                        in1=v[:, :, kk, :], op=mybir.AluOpType.add)

                def dest(ot, qpos=qpos, q=q):
                    nc.sync.dma_start(out=fxo[:, qpos * 8:(qpos + q) * 8],
                                      in_=ot[:, :q * 8])
                affine_store(acc[:], q, dest)

            # dense streaming first (independent of all gathers)
            for sm in segs:
                if sm["mode"] == "dense":
                    emit_pb(sm)
            # per-level pipeline
            qpos_map = {}
            qpos = 0
            for sm in segs:
                qpos_map[sm["li"]] = qpos
                qpos += sm["q"]
            for sm in segs:
                if sm["ntiles"]:
                    emit_pa(sm)
                if sm["mode"] != "dense":
                    emit_pb(sm)
                emit_pc(sm, qpos_map[sm["li"]])

            # ---- hard fixups (rare; neighbors cross levels) ----
            if HQ:
                nh = HQ // P
                hi = pb.tile([P, nh * 8], i32, tag="hi")
                nc.sync.dma_start(out=hi[:], in_=hrowst.ap().rearrange(
                    "(a p) b -> p (a b)", p=P))
                hacc = pb.tile([P, nh * 8], f32, tag="hacc")
                hrow = pb.tile([P, 8], bf16, tag="hrow")
                for j in range(nh):
                    for kk in range(8):
                        nc.gpsimd.indirect_dma_start(
                            out=hrow[:], out_offset=None, in_=xb.ap(),
                            in_offset=bass.IndirectOffsetOnAxis(
                                ap=hi[:, j * 8 + kk:j * 8 + kk + 1], axis=0))
                        if kk == 0:
                            nc.vector.tensor_copy(out=hacc[:, j * 8:(j + 1) * 8], in_=hrow[:])
                        else:
                            nc.vector.tensor_tensor(
                                out=hacc[:, j * 8:(j + 1) * 8],
                                in0=hacc[:, j * 8:(j + 1) * 8], in1=hrow[:],
                                op=mybir.AluOpType.add)

                def hdest(ot, nh=nh):
                    nc.sync.dma_start(out=fxo[:, NQS * 8:(NQS + nh) * 8],
                                      in_=ot[:, :nh * 8])
                affine_store(hacc[:], nh, hdest)
    nc.compile()
    return nc


def kernel(x, W, b, neighbor_idx):
    x = np.asarray(x)
    W = np.asarray(W, dtype=np.float32)
    b = np.asarray(b, dtype=np.float32)
    neighbor_idx = np.asarray(neighbor_idx, dtype=np.int64)
    in_dtype = x.dtype
    x2 = x.reshape(x.shape[0], -1).astype(np.float32)
    N = x2.shape[0]

    plan = _plan(neighbor_idx)
    segs = plan["segs"]
    tb = 0
    for sm in segs:
        sm["tile_base"] = tb
        tb += sm["ntiles"]
        if sm["mode"] == "dense":
            sm["es8"] = 0  # per-core entry start handled via xd slice offset
    NT = tb

    # bf16 table (round-to-nearest-ish)
    xb_bits = ((x2.view(np.uint32) + 0x8000) >> 16).astype(np.uint16)
    xbf = xb_bits.view(_bf16).reshape(N, 8)
    dense_end = max(sm["off"] + plan["levels"][sm["li"]]["T"] for sm in segs if sm["mode"] == "dense")
    dense_rows = min(N, dense_end + 300000)

    per_core = []
    NQS = sum(sm["q"] for sm in segs)
    for c in range(NC):
        gidx, msk, scat, hrows, hout = _core_arrays(plan, neighbor_idx, c)
        # dense phase-B reads xd at per-core offsets: we shift the dense data
        # per core instead (xd differs per core)
        xd = np.zeros((dense_rows, 8), dtype=np.float32)
        for sm in segs:
            if sm["mode"] != "dense":
                continue
            lv = plan["levels"][sm["li"]]
            es = c * sm["chunk"]
            lo = lv["off"] + es
            hi = min(N, lo + P * sm["PL"] + sm["R"] ** 2 + sm["R"] + 2)
            xd[lv["off"]:lv["off"] + (hi - lo)] = x2[lo:hi]
        wt = np.tile((W / 8.0).reshape(1, 64), (P, 1)).astype(np.float32)
        bt = np.tile(b.reshape(1, 8), (P, 1)).astype(np.float32)
        m = dict(xb=np.ascontiguousarray(xbf), xd=xd, gidx=gidx, mskd=msk,
                 wt=wt, bt=bt, scat=scat)
        if plan["HQ"]:
            m["hrows"] = hrows[:plan["HQ"]]
            m["hout"] = hout[:plan["HQ"]]
        per_core.append(m)

    nc = _build_nc(plan, NT, NQS, dense_rows)
    kernel.last_nc = nc
    kernel.last_per_core = per_core
    import time as _time
    _t0 = _time.time()
    res = run_bass_kernel_spmd(nc, per_core, list(range(NC)))
    kernel.last_spmd_wall_s = _time.time() - _t0

    full = np.empty((N, 8), dtype=np.float32)
    for c in range(NC):
        co = res.results[c]["out"]
        for sm in segs:
            lv = plan["levels"][sm["li"]]
            es = c * sm["chunk"]
            ecount = min(sm["chunk"], lv["T"] - es)
            if ecount <= 0:
                continue
            full[lv["off"] + es: lv["off"] + es + ecount] = co[sm["o0"]: sm["o0"] + ecount]
    # overlay device-computed fixup rows (host does placement only)
    for c in range(NC):
        fx = res.results[c]["fxo"]
        qpos = 0
        for sm in segs:
            lv = plan["levels"][sm["li"]]
            q = sm["q"]
            fxs = plan["fix"][c][sm["li"]]
            for f, j in enumerate(fxs):
                p, jj = f // q, f % q
                full[lv["off"] + j] = fx[p, (qpos + jj) * 8:(qpos + jj + 1) * 8]
            qpos += q
        if plan["HQ"]:
            nh = plan["HQ"] // P
            for f, ge in enumerate(plan["hard"][c]):
                p, jj = f % P, f // P
                full[ge] = fx[p, (NQS + jj) * 8:(NQS + jj + 1) * 8]
    return full.reshape(x.shape).astype(in_dtype)
